# revision 1
# baseline (speedup 1.0000x reference)
"""Trainium2 Bass kernel for a 2-layer BiLSTM text tagger.

Model (see reference): embedding gather -> BiLSTM(128) -> BiLSTM(128) with
residual -> dense(279) -> softmax. mask_zero=True semantics (state + output
carry-through at masked steps).

Sharding: data-parallel over batch, 4 examples per core on 8 cores. Each core
runs the full network for its slice; no collectives.

Device layout (per core, "layout B" — feature/gate dim on partitions, batch in
the free dim):
  XT[k]  [128, 2048] bf16  - gathered embeddings, transposed; feature = 128k+p,
                             token col j = 4t+e (t-major, e = local example)
  Zb     [128, 16384] bf16 - input projections in PSUM-bank order:
                             col = 32s + 16d + 4c + e (s step, d dir, c gate
                             chunk i/f/g/o, e example). g-chunk pre-scaled by 2
                             so one Sigmoid over all 32 cols computes i,f,o
                             sigmoids and sigma(2 z_g) (tanh via 2*sig(2x)-1).
  H*     [128, 2048] bf16  - hidden states, col = 4t + e
  Recurrence step: one identity-matmul injects 16 steps of Z into a PSUM bank
  (start=True), then per step 8 accumulating matmuls add h @ Wr per
  (dir, gate-chunk); Sigmoid reads the 32-col slice; DVE computes the cell
  update with a fused scalar_tensor_tensor for the tanh fix-up.
"""

import json

import ml_dtypes
import numpy as np

# ---------------------------------------------------------------------------
# problem constants (hardcoded per the contract)
B, T = 32, 512
EMB, UNITS, NCLS = 300, 128, 279
VOCAB = 100000
NCORES = 8
BL = B // NCORES          # 4 examples / core
NTOK = BL * T             # 2048 tokens / core
G4 = 4 * UNITS            # 512
KPAD = 384                # padded embedding dim (3 x 128)
NU = 2048                 # compact table rows (fixed shape across cores)
NTILE = NTOK // 128       # 16 token tiles

_prog_cache = {}


# ---------------------------------------------------------------------------
def _apply_bir_wait_split(bass_mod):
    """This container's walrus rejects >1 sync-wait per instruction. Split
    extras onto inserted EventSemaphore instructions (same engine, in order).
    """
    if getattr(bass_mod.Bass, "_wait_split_applied", False):
        return
    orig = bass_mod.Bass.to_json_bytes
    ctr = [0]

    def fix_list(lst):
        out, changed = [], False
        for ins in lst:
            si = ins.get("sync_info") if isinstance(ins, dict) else None
            if not si:
                out.append(ins)
                continue
            waits = si.get("on_wait") or []
            upds = si.get("on_update") or []
            if len(waits) > 1:
                for w in waits[1:]:
                    ctr[0] += 1
                    out.append({
                        "debug": ins.get("debug", 0), "engine": ins["engine"],
                        "ins": [], "name": f"I-waitfix-{ctr[0]}",
                        "opcode": "EventSemaphore", "outs": [],
                        "sync_info": {"on_update": [], "on_wait": [w]},
                    })
                si["on_wait"] = waits[:1]
                changed = True
            out.append(ins)
            if len(upds) > 1:
                for u in upds[1:]:
                    ctr[0] += 1
                    out.append({
                        "debug": ins.get("debug", 0), "engine": ins["engine"],
                        "ins": [], "name": f"I-updfix-{ctr[0]}",
                        "opcode": "EventSemaphore", "outs": [],
                        "sync_info": {"on_update": [u], "on_wait": []},
                    })
                si["on_update"] = upds[:1]
                changed = True
        return out, changed

    def walk(o):
        if isinstance(o, dict):
            for k, v in o.items():
                if (isinstance(v, list) and v
                        and all(isinstance(e, dict) and "opcode" in e for e in v)):
                    fixed, changed = fix_list(v)
                    if changed:
                        o[k] = fixed
                    for e in o[k]:
                        walk(e)
                else:
                    walk(v)
        elif isinstance(o, list):
            for v in o:
                walk(v)

    def to_json_bytes_fixed(self):
        d = json.loads(orig(self))
        walk(d)
        return json.dumps(d).encode()

    bass_mod.Bass.to_json_bytes = to_json_bytes_fixed
    bass_mod.Bass._wait_split_applied = True


# ---------------------------------------------------------------------------
def _build_program(mask_entries, has_clsb, phases='full', variant=4):
    """Build the Bass program (shared by all 8 cores).

    mask_entries: sorted tuple of (d, s) recurrence slots that need the
    data-driven carry-through lerp (d: 0 fwd / 1 bwd, s: step index).
    """
    import concourse.bass as bass
    import concourse.mybir as mybir
    import concourse.tile as tile

    _apply_bir_wait_split(bass)

    bf16 = mybir.dt.bfloat16
    f32 = mybir.dt.float32
    i32 = mybir.dt.int32
    AF = mybir.ActivationFunctionType
    ALU = mybir.AluOpType

    nc = bass.Bass()

    # ---- DRAM I/O ----
    tbl = nc.dram_tensor("tbl", [NU, KPAD], bf16, kind="ExternalInput")
    idx = nc.dram_tensor("idx", [128, NTILE], i32, kind="ExternalInput")
    ident_d = nc.dram_tensor("ident", [128, 128], bf16, kind="ExternalInput")
    w0_d = nc.dram_tensor("w0", [2, 3, 128, G4], bf16, kind="ExternalInput")
    r0_d = nc.dram_tensor("r0", [2, 128, G4], bf16, kind="ExternalInput")
    w1_d = nc.dram_tensor("w1", [2, 2, 128, G4], bf16, kind="ExternalInput")
    r1_d = nc.dram_tensor("r1", [2, 128, G4], bf16, kind="ExternalInput")
    b0_d = nc.dram_tensor("b0", [128, 8], f32, kind="ExternalInput")
    b1_d = nc.dram_tensor("b1", [128, 8], f32, kind="ExternalInput")
    clsw_d = nc.dram_tensor("clsw", [2, 128, NCLS], bf16, kind="ExternalInput")
    nmask = max(1, len(mask_entries))
    msk_d = nc.dram_tensor("msk", [128, 4 * nmask], f32, kind="ExternalInput")
    clsb_d = None
    if has_clsb:
        clsb_d = nc.dram_tensor("clsb", [128, NCLS], f32, kind="ExternalInput")
    out_d = nc.dram_tensor("out", [NTOK, NCLS], f32, kind="ExternalOutput")

    mask_idx = {ds: i for i, ds in enumerate(mask_entries)}

    with tile.TileContext(nc) as tc:
        with (
            tc.tile_pool(name="const", bufs=1) as cpool,
            tc.tile_pool(name="big", bufs=1) as bigpool,
            tc.tile_pool(name="state", bufs=1) as spool,
        ):
            # ---- constants to SBUF ----
            idx_sb = cpool.tile([128, NTILE], i32)
            nc.gpsimd.dma_start(out=idx_sb[:, :], in_=idx[:, :])
            ident = cpool.tile([128, 128], bf16)
            nc.gpsimd.dma_start(out=ident[:, :], in_=ident_d[:, :])
            w0 = cpool.tile([128, 2, 3, G4], bf16)
            nc.gpsimd.dma_start(
                out=w0[:, :, :, :], in_=w0_d.rearrange("d k p g -> p d k g"))
            r0 = cpool.tile([128, 2, G4], bf16)
            nc.gpsimd.dma_start(out=r0[:, :, :], in_=r0_d.rearrange("d p g -> p d g"))
            w1 = cpool.tile([128, 2, 2, G4], bf16)
            nc.gpsimd.dma_start(
                out=w1[:, :, :, :], in_=w1_d.rearrange("d k p g -> p d k g"))
            r1 = cpool.tile([128, 2, G4], bf16)
            nc.gpsimd.dma_start(out=r1[:, :, :], in_=r1_d.rearrange("d p g -> p d g"))
            b0 = cpool.tile([128, 8], f32)
            nc.gpsimd.dma_start(out=b0[:, :], in_=b0_d[:, :])
            b1 = cpool.tile([128, 8], f32)
            nc.gpsimd.dma_start(out=b1[:, :], in_=b1_d[:, :])
            clsw = cpool.tile([128, 2, NCLS], bf16)
            nc.gpsimd.dma_start(out=clsw[:, :, :], in_=clsw_d.rearrange("k p n -> p k n"))
            msk = cpool.tile([128, 4 * nmask], f32)
            nc.gpsimd.dma_start(out=msk[:, :], in_=msk_d[:, :])
            clsb = None
            if has_clsb:
                clsb = cpool.tile([128, NCLS], f32)
                nc.gpsimd.dma_start(out=clsb[:, :], in_=clsb_d[:, :])

            # ---- big persistent buffers ----
            xt = [bigpool.tile([128, NTOK], bf16, tag=f"xt{k}", name=f"xt{k}")
                  for k in range(3)]
            zb = bigpool.tile([128, 32 * T], bf16)
            h0f = bigpool.tile([128, NTOK], bf16)
            h0b = bigpool.tile([128, NTOK], bf16)
            h1f = bigpool.tile([128, NTOK], bf16)
            h1b = bigpool.tile([128, NTOK], bf16)

            hz = spool.tile([128, 8], bf16)
            nc.vector.memset(hz[:, :], 0.0)

            def strided(tileap, offset, dims):
                return bass.AP(tensor=tileap.tensor, offset=tileap.offset + offset,
                               ap=[tileap.ap[0]] + dims)

            # ================= Phase A: gather + transpose =================
            with (
                tc.tile_pool(name="xrow", bufs=4) as xrow_pool,
                tc.tile_pool(name="tpps", bufs=4, space="PSUM") as tp_pool,
            ):
                for c in range(NTILE):
                    xrow = xrow_pool.tile([128, KPAD], bf16, tag="xrow")
                    nc.gpsimd.indirect_dma_start(
                        out=xrow[:, :], out_offset=None, in_=tbl[:, :],
                        in_offset=bass.IndirectOffsetOnAxis(
                            ap=idx_sb[:, c:c + 1], axis=0),
                    )
                    for k in range(3):
                        pst = tp_pool.tile([128, 128], bf16, tag="tp")
                        nc.tensor.transpose(
                            out=pst[:, :], in_=xrow[:, k * 128:(k + 1) * 128],
                            identity=ident[:, :])
                        nc.vector.tensor_copy(
                            xt[k][:, c * 128:(c + 1) * 128], pst[:, :])

            # ================= shared phase helpers =================
            def projection(layer):
                """Compute Zb for `layer` from its inputs (XT or H0)."""
                w = w0 if layer == 0 else w1
                bia = b0 if layer == 0 else b1
                nk = 3 if layer == 0 else 2
                with tc.tile_pool(name=f"pj{layer}", bufs=4, space="PSUM") as pjp:
                    for d in range(2):
                        for c in range(4):
                            for nb in range(4):
                                ps = pjp.tile([128, 512], f32, tag="pj")
                                s0 = 128 * nb
                                for k in range(nk):
                                    if layer == 0:
                                        src = xt[k][:, :]
                                    else:
                                        src = (h0f if k == 0 else h0b)[:, :]
                                    if d == 0:
                                        rhs = strided(src, 4 * s0,
                                                      [[4, 128], [1, 4]])
                                    else:
                                        rhs = strided(src, 4 * (511 - s0),
                                                      [[-4, 128], [1, 4]])
                                    nc.tensor.matmul(
                                        ps[:, :],
                                        w[:, d, k, c * 128:(c + 1) * 128],
                                        rhs, start=(k == 0), stop=(k == nk - 1))
                                dst = strided(zb[:, :], 32 * s0 + 16 * d + 4 * c,
                                              [[32, 128], [1, 4]])
                                nc.scalar.activation(
                                    dst, ps[:, :], AF.Identity,
                                    bias=bia[:, 4 * d + c:4 * d + c + 1], scale=1.0)

            def recurrence(layer):
                r = r0 if layer == 0 else r1
                Hf = h0f if layer == 0 else h1f
                Hb = h0b if layer == 0 else h1b
                with (
                    tc.tile_pool(name=f"rc{layer}", bufs=4 if variant == 0 else 6,
                                 space="PSUM") as rcp,
                    tc.tile_pool(name=f"gt{layer}", bufs=4 if variant == 0 else 8) as gtp,
                    tc.tile_pool(name=f"tm{layer}", bufs=3 if variant == 0 else 8) as tmp,
                ):
                    c_state = spool.tile([128, 8], f32, tag=f"c{layer}")
                    nc.vector.memset(c_state[:, :], 0.0)
                    ps = None
                    prev_ht = None
                    for s in range(T):
                        sb = s % 16
                        if sb == 0:
                            ps = rcp.tile([128, 512], f32, tag="bank")
                            nc.tensor.matmul(
                                ps[:, :], ident[:, :],
                                zb[:, 512 * (s // 16):512 * (s // 16) + 512],
                                start=True, stop=False, skip_group_check=True)
                        for d in range(2):
                            if s == 0:
                                hprev = hz[:, 4 * d:4 * d + 4]
                            elif variant >= 4 and prev_ht is not None:
                                hprev = prev_ht[:, 4 * d:4 * d + 4]
                            elif d == 0:
                                hprev = Hf[:, 4 * (s - 1):4 * (s - 1) + 4]
                            else:
                                hprev = Hb[:, 4 * (512 - s):4 * (512 - s) + 4]
                            for c in range(4):
                                nc.tensor.matmul(
                                    ps[:, 32 * sb + 16 * d + 4 * c:
                                       32 * sb + 16 * d + 4 * c + 4],
                                    r[:, d, c * 128:(c + 1) * 128],
                                    hprev, start=False, stop=False,
                                    skip_group_check=True)
                        sg = gtp.tile([128, 32], f32, tag="sg")
                        nc.scalar.activation(
                            sg[:, :], ps[:, 32 * sb:32 * sb + 32], AF.Sigmoid)
                        sga = sg[:, :]
                        i_ap = strided(sga, 0, [[16, 2], [1, 4]])
                        f_ap = strided(sga, 4, [[16, 2], [1, 4]])
                        g_ap = strided(sga, 8, [[16, 2], [1, 4]])
                        # u = i*g' ; w = 2u - i ; v = f*c ; c = v + w
                        if variant >= 3:
                            # i*(2g'-1) = 2*i*(g'-0.5): one fused op, then the
                            # *2 folds into the final accumulate.
                            w_t = tmp.tile([128, 8], f32, tag="w")
                            nc.vector.scalar_tensor_tensor(
                                out=w_t[:, :], in0=g_ap, scalar=0.5, in1=i_ap,
                                op0=ALU.subtract, op1=ALU.mult)
                        else:
                            ueng = nc.gpsimd if variant >= 2 else nc.vector
                            u = tmp.tile([128, 8], f32, tag="u")
                            ueng.tensor_tensor(
                                out=u[:, :], in0=i_ap, in1=g_ap, op=ALU.mult)
                            w_t = tmp.tile([128, 8], f32, tag="w")
                            ueng.scalar_tensor_tensor(
                                out=w_t[:, :], in0=u[:, :], scalar=2.0, in1=i_ap,
                                op0=ALU.mult, op1=ALU.subtract)
                        v = tmp.tile([128, 8], f32, tag="v")
                        nc.vector.tensor_tensor(
                            out=v[:, :], in0=f_ap, in1=c_state[:, :], op=ALU.mult)
                        masked = [d for d in range(2) if (d, s) in mask_idx]
                        if not masked:
                            if variant >= 3:
                                nc.vector.scalar_tensor_tensor(
                                    out=c_state[:, :], in0=w_t[:, :], scalar=2.0,
                                    in1=v[:, :], op0=ALU.mult, op1=ALU.add)
                            else:
                                nc.vector.tensor_tensor(
                                    out=c_state[:, :], in0=v[:, :], in1=w_t[:, :],
                                    op=ALU.add)
                            th = tmp.tile([128, 8], f32, tag="th")
                            nc.scalar.activation(th[:, :], c_state[:, :], AF.Tanh)
                            if variant >= 4:
                                o_ap = strided(sga, 12, [[16, 2], [1, 4]])
                                ht = tmp.tile([128, 8], bf16, tag="ht")
                                nc.vector.tensor_tensor(
                                    out=ht[:, :], in0=o_ap, in1=th[:, :],
                                    op=ALU.mult)
                                nc.vector.tensor_copy(
                                    Hf[:, 4 * s:4 * s + 4], ht[:, 0:4])
                                nc.vector.tensor_copy(
                                    Hb[:, 4 * (511 - s):4 * (511 - s) + 4],
                                    ht[:, 4:8])
                                prev_ht = ht
                            else:
                                nc.vector.tensor_tensor(
                                    out=Hf[:, 4 * s:4 * s + 4], in0=sg[:, 12:16],
                                    in1=th[:, 0:4], op=ALU.mult)
                                nc.vector.tensor_tensor(
                                    out=Hb[:, 4 * (511 - s):4 * (511 - s) + 4],
                                    in0=sg[:, 28:32], in1=th[:, 4:8], op=ALU.mult)
                        else:
                            cc = tmp.tile([128, 8], f32, tag="cc")
                            if variant >= 3:
                                nc.vector.scalar_tensor_tensor(
                                    out=cc[:, :], in0=w_t[:, :], scalar=2.0,
                                    in1=v[:, :], op0=ALU.mult, op1=ALU.add)
                            else:
                                nc.vector.tensor_tensor(
                                    out=cc[:, :], in0=v[:, :], in1=w_t[:, :], op=ALU.add)
                            # c lerp: cc_d = c_old + m*(cc_d - c_old)
                            for d in masked:
                                mi = mask_idx[(d, s)]
                                mcol = msk[:, 4 * mi:4 * mi + 4]
                                dd = tmp.tile([128, 4], f32, tag="dd")
                                nc.vector.tensor_tensor(
                                    out=dd[:, :], in0=cc[:, 4 * d:4 * d + 4],
                                    in1=c_state[:, 4 * d:4 * d + 4], op=ALU.subtract)
                                nc.vector.tensor_tensor(
                                    out=dd[:, :], in0=dd[:, :], in1=mcol, op=ALU.mult)
                                nc.vector.tensor_tensor(
                                    out=cc[:, 4 * d:4 * d + 4], in0=dd[:, :],
                                    in1=c_state[:, 4 * d:4 * d + 4], op=ALU.add)
                            nc.vector.tensor_copy(c_state[:, :], cc[:, :])
                            th = tmp.tile([128, 8], f32, tag="th")
                            nc.scalar.activation(th[:, :], c_state[:, :], AF.Tanh)
                            for d in range(2):
                                o_sl = sg[:, 16 * d + 12:16 * d + 16]
                                th_sl = th[:, 4 * d:4 * d + 4]
                                dst = (Hf[:, 4 * s:4 * s + 4] if d == 0 else
                                       Hb[:, 4 * (511 - s):4 * (511 - s) + 4])
                                if d in masked:
                                    mi = mask_idx[(d, s)]
                                    mcol = msk[:, 4 * mi:4 * mi + 4]
                                    if s == 0:
                                        hp = hz[:, 4 * d:4 * d + 4]
                                    elif d == 0:
                                        hp = Hf[:, 4 * (s - 1):4 * (s - 1) + 4]
                                    else:
                                        hp = Hb[:, 4 * (512 - s):4 * (512 - s) + 4]
                                    hn = tmp.tile([128, 4], f32, tag="hn")
                                    nc.vector.tensor_tensor(
                                        out=hn[:, :], in0=o_sl, in1=th_sl,
                                        op=ALU.mult)
                                    nc.vector.tensor_tensor(
                                        out=hn[:, :], in0=hn[:, :], in1=hp,
                                        op=ALU.subtract)
                                    nc.vector.tensor_tensor(
                                        out=hn[:, :], in0=hn[:, :], in1=mcol,
                                        op=ALU.mult)
                                    nc.vector.tensor_tensor(
                                        out=dst, in0=hn[:, :], in1=hp, op=ALU.add)
                                else:
                                    nc.vector.tensor_tensor(
                                        out=dst, in0=o_sl, in1=th_sl, op=ALU.mult)
                            prev_ht = None

            # ================= run the phases =================
            if phases in ('B', 'C', 'full'):
                projection(0)
            if phases in ('C', 'full'):
                recurrence(0)
            if phases == 'full':
                projection(1)
                recurrence(1)

            # ================= classifier + softmax =================
            with (
                tc.tile_pool(name="cls", bufs=4) as clp,
                tc.tile_pool(name="clps", bufs=4, space="PSUM") as clps,
            ):
                for tt in range(NTILE if phases == 'full' else 1):
                    sl = slice(128 * tt, 128 * (tt + 1))
                    i0 = clp.tile([128, 128], bf16, tag="i0")
                    nc.vector.tensor_tensor(
                        out=i0[:, :], in0=h0f[:, sl], in1=h1f[:, sl], op=ALU.add)
                    i1 = clp.tile([128, 128], bf16, tag="i1")
                    nc.vector.tensor_tensor(
                        out=i1[:, :], in0=h0b[:, sl], in1=h1b[:, sl], op=ALU.add)
                    pc = clps.tile([128, NCLS], f32, tag="pc")
                    nc.tensor.matmul(pc[:, :], i0[:, :], clsw[:, 0, :],
                                     start=True, stop=False)
                    nc.tensor.matmul(pc[:, :], i1[:, :], clsw[:, 1, :],
                                     start=False, stop=True)
                    ex = clp.tile([128, NCLS], f32, tag="ex")
                    if has_clsb:
                        nc.vector.tensor_tensor(
                            out=ex[:, :], in0=pc[:, :], in1=clsb[:, :], op=ALU.add)
                        nc.scalar.activation(ex[:, :], ex[:, :], AF.Exp)
                    else:
                        nc.scalar.activation(ex[:, :], pc[:, :], AF.Exp)
                    ssum = clp.tile([128, 1], f32, tag="ss")
                    nc.vector.tensor_reduce(
                        out=ssum[:, :], in_=ex[:, :], op=ALU.add,
                        axis=mybir.AxisListType.X)
                    rec_t = clp.tile([128, 1], f32, tag="rc")
                    nc.vector.reciprocal(rec_t[:, :], ssum[:, :])
                    sm = clp.tile([128, NCLS], f32, tag="sm")
                    nc.vector.tensor_scalar_mul(sm[:, :], ex[:, :], rec_t[:, :])
                    nc.gpsimd.dma_start(out=out_d[sl, :], in_=sm[:, :])

    return nc


# ---------------------------------------------------------------------------
def _prep_host(inputs):
    """Shard + pre-arrange all device inputs. Returns (in_maps, mask_entries,
    has_clsb)."""
    ids = np.asarray(inputs["ids"])
    emb = np.asarray(inputs["emb_table"], dtype=np.float32)

    def gate2(wk):
        w = np.array(wk, dtype=np.float32, copy=True)
        w[:, 2 * UNITS:3 * UNITS] *= 2.0
        return w

    def pad_k(w, kpad):
        out = np.zeros((kpad, G4), np.float32)
        out[:w.shape[0]] = w
        return out

    w0 = np.stack([
        pad_k(gate2(inputs["fw0_k"]), KPAD).reshape(3, 128, G4),
        pad_k(gate2(inputs["bw0_k"]), KPAD).reshape(3, 128, G4),
    ]).astype(ml_dtypes.bfloat16)
    r0 = np.stack([gate2(inputs["fw0_r"]), gate2(inputs["bw0_r"])]
                  ).astype(ml_dtypes.bfloat16)
    w1 = np.stack([
        gate2(inputs["fw1_k"]).reshape(2, 128, G4),
        gate2(inputs["bw1_k"]).reshape(2, 128, G4),
    ]).astype(ml_dtypes.bfloat16)
    r1 = np.stack([gate2(inputs["fw1_r"]), gate2(inputs["bw1_r"])]
                  ).astype(ml_dtypes.bfloat16)

    def bias_tile(bf, bb):
        out = np.zeros((128, 8), np.float32)
        for d, b in enumerate((bf, bb)):
            b = np.array(b, dtype=np.float32, copy=True)
            b[2 * UNITS:3 * UNITS] *= 2.0
            out[:, 4 * d:4 * d + 4] = b.reshape(4, 128).T
        return out

    b0 = bias_tile(inputs["fw0_b"], inputs["bw0_b"])
    b1 = bias_tile(inputs["fw1_b"], inputs["bw1_b"])
    clsw = np.asarray(inputs["cls_w"], np.float32).reshape(2, 128, NCLS).astype(
        ml_dtypes.bfloat16)
    clsb_np = np.asarray(inputs["cls_b"], np.float32)
    has_clsb = bool(np.any(clsb_np != 0))
    ident = np.eye(128, dtype=ml_dtypes.bfloat16)

    mask_entry_set = set()
    per_core = []
    for c in range(NCORES):
        ids_c = ids[BL * c:BL * (c + 1)].astype(np.int64)      # [BL, T]
        ids_tm = ids_c.T.reshape(-1)                           # j = t*BL + e
        uniq, inv = np.unique(ids_tm, return_inverse=True)
        tblp = np.zeros((NU, KPAD), ml_dtypes.bfloat16)
        tblp[:len(uniq), :EMB] = emb[uniq].astype(ml_dtypes.bfloat16)
        idx_np = inv.astype(np.int32).reshape(NTILE, 128).T.copy()
        mask_c = (ids_c != 0)
        for e, t in zip(*np.nonzero(~mask_c)):
            mask_entry_set.add((0, int(t)))          # fwd step s = t
            mask_entry_set.add((1, int(511 - t)))    # bwd step s = 511 - t
        per_core.append((tblp, idx_np, mask_c))

    mask_entries = tuple(sorted(mask_entry_set))
    nmask = max(1, len(mask_entries))

    in_maps = []
    for c in range(NCORES):
        tblp, idx_np, mask_c = per_core[c]
        msk = np.ones((128, 4 * nmask), np.float32)
        for mi, (d, s) in enumerate(mask_entries):
            t = s if d == 0 else 511 - s
            msk[:, 4 * mi:4 * mi + 4] = mask_c[:, t].astype(np.float32)[None, :]
        m = dict(tbl=tblp, idx=idx_np, ident=ident, w0=w0, r0=r0, w1=w1, r1=r1,
                 b0=b0, b1=b1, clsw=clsw, msk=msk)
        if has_clsb:
            m["clsb"] = np.broadcast_to(
                clsb_np.astype(np.float32), (128, NCLS)).copy()
        in_maps.append(m)
    return in_maps, mask_entries, has_clsb


# ---------------------------------------------------------------------------
def kernel(**inputs):
    from concourse.bass_utils import run_bass_kernel_spmd

    in_maps, mask_entries, has_clsb = _prep_host(inputs)

    key = (mask_entries, has_clsb)
    if key not in _prog_cache:
        _prog_cache[key] = _build_program(mask_entries, has_clsb)
    nc = _prog_cache[key]

    res = run_bass_kernel_spmd(nc, in_maps, core_ids=list(range(NCORES)))

    out = np.empty((B, T, NCLS), np.float32)
    for c in range(NCORES):
        oc = res.results[c]["out"].reshape(T, BL, NCLS)
        out[BL * c:BL * (c + 1)] = oc.transpose(1, 0, 2)
    return out



# revision 4
# speedup vs baseline: 3.9859x; 3.9859x over previous
"""Trainium2 Bass kernel for a 2-layer BiLSTM text tagger.

Model (see reference): embedding gather -> BiLSTM(128) -> BiLSTM(128) with
residual -> dense(279) -> softmax. mask_zero=True semantics (state + output
carry-through at masked steps).

Sharding: data-parallel over batch, 4 examples per core on 8 cores.

The per-call cost in this environment is dominated by host->device transfer
over the axon relay (~50-90 MB/s, ~80 ms dispatch floor), so the I/O layout
is optimized for minimum bytes on the wire:
  - the embedding gather + transpose happens on HOST; the device receives
    xq [300, 2048] int8 (per-feature-scaled) per core (~0.6 MB/core),
    dequantized to bf16 on device with one DVE op per 128-feature chunk.
  - all weights (layer0/1 kernels+recurrent, classifier, identity) are
    packed into ONE [128, CTOT] bf16 "wall"; each core receives 1/8 of it
    (16 rows) and an on-device DRAM AllGather reconstructs the full wall
    (2 MB total over the wire instead of 16 MB replicated).
  - biases/masks ride in one small f32 [128, NB] tile.
  - donated output zero-buffers are created on device (jnp.zeros inside the
    jitted body), not transferred.

Device layout (per core, feature/gate dim on partitions, batch in free dim):
  XT[k]  [128, 2048] bf16  - embeddings, feature = 128k+p, col j = 4t+e
  Zb     [128, 16384] bf16 - input projections in PSUM-bank order:
                             col = 32s + 16d + 4c + e (s step, d dir, c gate
                             chunk i/f/g/o, e example). g-chunk pre-scaled by
                             2 so one Sigmoid computes i,f,o sigmoids and
                             sigma(2 z_g) (tanh via 2*sig(2x)-1).
  H*     [128, 2048] bf16  - hidden states, col = 4t + e
  Recurrence step: one identity-matmul injects 16 steps of Z into a PSUM
  bank (start=True), then per step 8 accumulating matmuls add h @ Wr per
  (dir, gate-chunk); Sigmoid reads the 32-col slice; DVE computes the cell
  update with a fused scalar_tensor_tensor for the tanh fix-up.
"""

import json

import ml_dtypes
import numpy as np

# ---------------------------------------------------------------------------
# problem constants (hardcoded per the contract)
B, T = 32, 512
EMB, UNITS, NCLS = 300, 128, 279
VOCAB = 100000
NCORES = 8
BL = B // NCORES          # 4 examples / core
NTOK = BL * T             # 2048 tokens / core
G4 = 4 * UNITS            # 512
NTILE = NTOK // 128       # 16 token tiles

# wall (packed weights) column layout, bf16
OFF_W0 = 0                # [128, 2, 3, 512]  (k=2 chunk rows >=44 are zero)
OFF_R0 = OFF_W0 + 2 * 3 * G4          # 3072
OFF_W1 = OFF_R0 + 2 * G4              # 4096
OFF_R1 = OFF_W1 + 2 * 2 * G4          # 6144
OFF_CLS = OFF_R1 + 2 * G4             # 7168, [128, 2, 279]
OFF_ID = OFF_CLS + 2 * NCLS           # 7726, [128, 128] identity
CTOT = OFF_ID + 128                   # 7854
WTOT = 128 * CTOT                     # 1,005,312 elems
WSH = WTOT // NCORES                  # 125,664 elems (= 16 rows)
SHROWS = 128 // NCORES                # 16 rows per shard

_prog_cache = {}
_fn_cache = {}


# ---------------------------------------------------------------------------
def _apply_bir_wait_split(bass_mod):
    """This container's walrus rejects >1 sync-wait per instruction. Split
    extras onto inserted EventSemaphore instructions (same engine, in order).
    """
    if getattr(bass_mod.Bass, "_wait_split_applied", False):
        return
    orig = bass_mod.Bass.to_json_bytes
    ctr = [0]

    def fix_list(lst):
        out, changed = [], False
        for ins in lst:
            si = ins.get("sync_info") if isinstance(ins, dict) else None
            if not si:
                out.append(ins)
                continue
            waits = si.get("on_wait") or []
            upds = si.get("on_update") or []
            if len(waits) > 1:
                for w in waits[1:]:
                    ctr[0] += 1
                    out.append({
                        "debug": ins.get("debug", 0), "engine": ins["engine"],
                        "ins": [], "name": f"I-waitfix-{ctr[0]}",
                        "opcode": "EventSemaphore", "outs": [],
                        "sync_info": {"on_update": [], "on_wait": [w]},
                    })
                si["on_wait"] = waits[:1]
                changed = True
            out.append(ins)
            if len(upds) > 1:
                for u in upds[1:]:
                    ctr[0] += 1
                    out.append({
                        "debug": ins.get("debug", 0), "engine": ins["engine"],
                        "ins": [], "name": f"I-updfix-{ctr[0]}",
                        "opcode": "EventSemaphore", "outs": [],
                        "sync_info": {"on_update": [u], "on_wait": []},
                    })
                si["on_update"] = upds[:1]
                changed = True
        return out, changed

    def walk(o):
        if isinstance(o, dict):
            for k, v in o.items():
                if (isinstance(v, list) and v
                        and all(isinstance(e, dict) and "opcode" in e for e in v)):
                    fixed, changed = fix_list(v)
                    if changed:
                        o[k] = fixed
                    for e in o[k]:
                        walk(e)
                else:
                    walk(v)
        elif isinstance(o, list):
            for v in o:
                walk(v)

    def to_json_bytes_fixed(self):
        d = json.loads(orig(self))
        walk(d)
        return json.dumps(d).encode()

    bass_mod.Bass.to_json_bytes = to_json_bytes_fixed
    bass_mod.Bass._wait_split_applied = True


# ---------------------------------------------------------------------------
def _build_program(mask_entries, has_clsb):
    """Build the Bass program (shared by all 8 cores).

    mask_entries: sorted tuple of (d, s) recurrence slots that need the
    data-driven carry-through lerp (d: 0 fwd / 1 bwd, s: step index).
    """
    import concourse.bass as bass
    import concourse.mybir as mybir
    import concourse.tile as tile

    _apply_bir_wait_split(bass)

    bf16 = mybir.dt.bfloat16
    f32 = mybir.dt.float32
    i8 = mybir.dt.int8
    AF = mybir.ActivationFunctionType
    ALU = mybir.AluOpType

    nc = bass.Bass(num_devices=NCORES)

    nmask = max(1, len(mask_entries))
    NB = 16 + 4 * nmask + (NCLS if has_clsb else 0)

    # ---- DRAM I/O ----
    xq_d = nc.dram_tensor("xq", [EMB, NTOK], i8, kind="ExternalInput")
    xs_d = nc.dram_tensor("xs", [128, 3], f32, kind="ExternalInput")
    wsh_d = nc.dram_tensor("wsh", [1, WSH], bf16, kind="ExternalInput")
    bb_d = nc.dram_tensor("bb", [128, NB], f32, kind="ExternalInput")
    out_d = nc.dram_tensor("out", [NTOK, NCLS], f32, kind="ExternalOutput")

    mask_idx = {ds: i for i, ds in enumerate(mask_entries)}

    with tile.TileContext(nc) as tc:
        with (
            tc.tile_pool(name="const", bufs=1) as cpool,
            tc.tile_pool(name="big", bufs=1) as bigpool,
            tc.tile_pool(name="state", bufs=1) as spool,
            tc.tile_pool(name="dram", bufs=1, space="DRAM") as dram,
        ):
            # ---- weight wall: shard -> DRAM AllGather -> SBUF ----
            inb = dram.tile([1, WSH], bf16)
            outb = dram.tile([128, CTOT], bf16)
            nc.gpsimd.dma_start(out=inb[:, :], in_=wsh_d[:, :])
            nc.gpsimd.collective_compute(
                "AllGather", mybir.AluOpType.bypass,
                replica_groups=[list(range(NCORES))],
                ins=[inb[:, :].opt()], outs=[outb[:, :].opt()],
            )
            wall = bigpool.tile([128, CTOT], bf16)
            nc.gpsimd.dma_start(out=wall[:, :], in_=outb[:, :])

            # ---- small constants ----
            xs_sb = cpool.tile([128, 3], f32)
            nc.gpsimd.dma_start(out=xs_sb[:, :], in_=xs_d[:, :])
            bb = cpool.tile([128, NB], f32)
            nc.gpsimd.dma_start(out=bb[:, :], in_=bb_d[:, :])

            # ---- embeddings: int8 -> bf16 dequant (per-feature scale) ----
            xt = []
            with tc.tile_pool(name="xqp", bufs=1) as xqp:
                for k in range(3):
                    nk = min(128, EMB - 128 * k)
                    xqt = xqp.tile([128, NTOK], i8, tag=f"xq{k}", name=f"xq{k}")
                    nc.gpsimd.dma_start(
                        out=xqt[:nk, :], in_=xq_d[128 * k:128 * k + nk, :])
                    xk = bigpool.tile([128, NTOK], bf16, tag=f"xt{k}",
                                      name=f"xt{k}")
                    if nk < 128:
                        nc.vector.memset(xk[:, :], 0.0)
                    nc.vector.tensor_scalar_mul(
                        xk[:nk, :], xqt[:nk, :], xs_sb[:nk, k:k + 1])
                    xt.append(xk)

            # ---- big persistent buffers ----
            zb = bigpool.tile([128, 32 * T], bf16)
            h0f = bigpool.tile([128, NTOK], bf16)
            h0b = bigpool.tile([128, NTOK], bf16)
            h1f = bigpool.tile([128, NTOK], bf16)
            h1b = bigpool.tile([128, NTOK], bf16)

            hz = spool.tile([128, 8], bf16)
            nc.vector.memset(hz[:, :], 0.0)

            def strided(tileap, offset, dims):
                return bass.AP(tensor=tileap.tensor, offset=tileap.offset + offset,
                               ap=[tileap.ap[0]] + dims)

            ident = wall[:, OFF_ID:OFF_ID + 128]

            # ================= shared phase helpers =================
            def projection(layer):
                """Compute Zb for `layer` from its inputs (XT or H0)."""
                nk = 3 if layer == 0 else 2
                woff = OFF_W0 if layer == 0 else OFF_W1
                boff = 0 if layer == 0 else 8
                with tc.tile_pool(name=f"pj{layer}", bufs=4, space="PSUM") as pjp:
                    for d in range(2):
                        for c in range(4):
                            for nb in range(4):
                                ps = pjp.tile([128, 512], f32, tag="pj")
                                s0 = 128 * nb
                                for k in range(nk):
                                    if layer == 0:
                                        src = xt[k][:, :]
                                    else:
                                        src = (h0f if k == 0 else h0b)[:, :]
                                    if d == 0:
                                        rhs = strided(src, 4 * s0,
                                                      [[4, 128], [1, 4]])
                                    else:
                                        rhs = strided(src, 4 * (511 - s0),
                                                      [[-4, 128], [1, 4]])
                                    wcol = woff + (d * nk + k) * G4 + c * 128
                                    nc.tensor.matmul(
                                        ps[:, :],
                                        wall[:, wcol:wcol + 128],
                                        rhs, start=(k == 0), stop=(k == nk - 1))
                                dst = strided(zb[:, :], 32 * s0 + 16 * d + 4 * c,
                                              [[32, 128], [1, 4]])
                                nc.scalar.activation(
                                    dst, ps[:, :], AF.Identity,
                                    bias=bb[:, boff + 4 * d + c:boff + 4 * d + c + 1],
                                    scale=1.0)

            def recurrence(layer):
                roff = OFF_R0 if layer == 0 else OFF_R1
                Hf = h0f if layer == 0 else h1f
                Hb = h0b if layer == 0 else h1b
                with (
                    tc.tile_pool(name=f"rc{layer}", bufs=6, space="PSUM") as rcp,
                    tc.tile_pool(name=f"gt{layer}", bufs=8) as gtp,
                    tc.tile_pool(name=f"tm{layer}", bufs=8) as tmp,
                ):
                    c_state = spool.tile([128, 8], f32, tag=f"c{layer}")
                    nc.vector.memset(c_state[:, :], 0.0)
                    ps = None
                    prev_ht = None
                    for s in range(T):
                        sb = s % 16
                        if sb == 0:
                            ps = rcp.tile([128, 512], f32, tag="bank")
                            nc.tensor.matmul(
                                ps[:, :], ident,
                                zb[:, 512 * (s // 16):512 * (s // 16) + 512],
                                start=True, stop=False, skip_group_check=True)
                        for d in range(2):
                            if s == 0:
                                hprev = hz[:, 4 * d:4 * d + 4]
                            elif prev_ht is not None:
                                hprev = prev_ht[:, 4 * d:4 * d + 4]
                            elif d == 0:
                                hprev = Hf[:, 4 * (s - 1):4 * (s - 1) + 4]
                            else:
                                hprev = Hb[:, 4 * (512 - s):4 * (512 - s) + 4]
                            for c in range(4):
                                rcol = roff + d * G4 + c * 128
                                nc.tensor.matmul(
                                    ps[:, 32 * sb + 16 * d + 4 * c:
                                       32 * sb + 16 * d + 4 * c + 4],
                                    wall[:, rcol:rcol + 128],
                                    hprev, start=False, stop=False,
                                    skip_group_check=True)
                        sg = gtp.tile([128, 32], f32, tag="sg")
                        nc.scalar.activation(
                            sg[:, :], ps[:, 32 * sb:32 * sb + 32], AF.Sigmoid)
                        sga = sg[:, :]
                        i_ap = strided(sga, 0, [[16, 2], [1, 4]])
                        f_ap = strided(sga, 4, [[16, 2], [1, 4]])
                        g_ap = strided(sga, 8, [[16, 2], [1, 4]])
                        # i*(2g'-1) = 2*i*(g'-0.5): one fused op; the *2 folds
                        # into the final accumulate.
                        w_t = tmp.tile([128, 8], f32, tag="w")
                        nc.vector.scalar_tensor_tensor(
                            out=w_t[:, :], in0=g_ap, scalar=0.5, in1=i_ap,
                            op0=ALU.subtract, op1=ALU.mult)
                        v = tmp.tile([128, 8], f32, tag="v")
                        nc.vector.tensor_tensor(
                            out=v[:, :], in0=f_ap, in1=c_state[:, :], op=ALU.mult)
                        masked = [d for d in range(2) if (d, s) in mask_idx]
                        if not masked:
                            nc.vector.scalar_tensor_tensor(
                                out=c_state[:, :], in0=w_t[:, :], scalar=2.0,
                                in1=v[:, :], op0=ALU.mult, op1=ALU.add)
                            th = tmp.tile([128, 8], f32, tag="th")
                            nc.scalar.activation(th[:, :], c_state[:, :], AF.Tanh)
                            o_ap = strided(sga, 12, [[16, 2], [1, 4]])
                            ht = tmp.tile([128, 8], bf16, tag="ht")
                            nc.vector.tensor_tensor(
                                out=ht[:, :], in0=o_ap, in1=th[:, :],
                                op=ALU.mult)
                            nc.vector.tensor_copy(
                                Hf[:, 4 * s:4 * s + 4], ht[:, 0:4])
                            nc.vector.tensor_copy(
                                Hb[:, 4 * (511 - s):4 * (511 - s) + 4],
                                ht[:, 4:8])
                            prev_ht = ht
                        else:
                            cc = tmp.tile([128, 8], f32, tag="cc")
                            nc.vector.scalar_tensor_tensor(
                                out=cc[:, :], in0=w_t[:, :], scalar=2.0,
                                in1=v[:, :], op0=ALU.mult, op1=ALU.add)
                            # c lerp: cc_d = c_old + m*(cc_d - c_old)
                            for d in masked:
                                mi = mask_idx[(d, s)]
                                mcol = bb[:, 16 + 4 * mi:16 + 4 * mi + 4]
                                dd = tmp.tile([128, 4], f32, tag="dd")
                                nc.vector.tensor_tensor(
                                    out=dd[:, :], in0=cc[:, 4 * d:4 * d + 4],
                                    in1=c_state[:, 4 * d:4 * d + 4], op=ALU.subtract)
                                nc.vector.tensor_tensor(
                                    out=dd[:, :], in0=dd[:, :], in1=mcol, op=ALU.mult)
                                nc.vector.tensor_tensor(
                                    out=cc[:, 4 * d:4 * d + 4], in0=dd[:, :],
                                    in1=c_state[:, 4 * d:4 * d + 4], op=ALU.add)
                            nc.vector.tensor_copy(c_state[:, :], cc[:, :])
                            th = tmp.tile([128, 8], f32, tag="th")
                            nc.scalar.activation(th[:, :], c_state[:, :], AF.Tanh)
                            for d in range(2):
                                o_sl = sg[:, 16 * d + 12:16 * d + 16]
                                th_sl = th[:, 4 * d:4 * d + 4]
                                dst = (Hf[:, 4 * s:4 * s + 4] if d == 0 else
                                       Hb[:, 4 * (511 - s):4 * (511 - s) + 4])
                                if d in masked:
                                    mi = mask_idx[(d, s)]
                                    mcol = bb[:, 16 + 4 * mi:16 + 4 * mi + 4]
                                    if s == 0:
                                        hp = hz[:, 4 * d:4 * d + 4]
                                    elif d == 0:
                                        hp = Hf[:, 4 * (s - 1):4 * (s - 1) + 4]
                                    else:
                                        hp = Hb[:, 4 * (512 - s):4 * (512 - s) + 4]
                                    hn = tmp.tile([128, 4], f32, tag="hn")
                                    nc.vector.tensor_tensor(
                                        out=hn[:, :], in0=o_sl, in1=th_sl,
                                        op=ALU.mult)
                                    nc.vector.tensor_tensor(
                                        out=hn[:, :], in0=hn[:, :], in1=hp,
                                        op=ALU.subtract)
                                    nc.vector.tensor_tensor(
                                        out=hn[:, :], in0=hn[:, :], in1=mcol,
                                        op=ALU.mult)
                                    nc.vector.tensor_tensor(
                                        out=dst, in0=hn[:, :], in1=hp, op=ALU.add)
                                else:
                                    nc.vector.tensor_tensor(
                                        out=dst, in0=o_sl, in1=th_sl, op=ALU.mult)
                            prev_ht = None

            # ================= run the phases =================
            projection(0)
            recurrence(0)
            projection(1)
            recurrence(1)

            # ================= classifier + softmax =================
            with (
                tc.tile_pool(name="cls", bufs=4) as clp,
                tc.tile_pool(name="clps", bufs=4, space="PSUM") as clps,
            ):
                for tt in range(NTILE):
                    sl = slice(128 * tt, 128 * (tt + 1))
                    i0 = clp.tile([128, 128], bf16, tag="i0")
                    nc.vector.tensor_tensor(
                        out=i0[:, :], in0=h0f[:, sl], in1=h1f[:, sl], op=ALU.add)
                    i1 = clp.tile([128, 128], bf16, tag="i1")
                    nc.vector.tensor_tensor(
                        out=i1[:, :], in0=h0b[:, sl], in1=h1b[:, sl], op=ALU.add)
                    pc = clps.tile([128, NCLS], f32, tag="pc")
                    nc.tensor.matmul(pc[:, :], i0[:, :],
                                     wall[:, OFF_CLS:OFF_CLS + NCLS],
                                     start=True, stop=False)
                    nc.tensor.matmul(pc[:, :], i1[:, :],
                                     wall[:, OFF_CLS + NCLS:OFF_CLS + 2 * NCLS],
                                     start=False, stop=True)
                    ex = clp.tile([128, NCLS], f32, tag="ex")
                    if has_clsb:
                        nc.vector.tensor_tensor(
                            out=ex[:, :], in0=pc[:, :],
                            in1=bb[:, NB - NCLS:NB], op=ALU.add)
                        nc.scalar.activation(ex[:, :], ex[:, :], AF.Exp)
                    else:
                        nc.scalar.activation(ex[:, :], pc[:, :], AF.Exp)
                    ssum = clp.tile([128, 1], f32, tag="ss")
                    nc.vector.tensor_reduce(
                        out=ssum[:, :], in_=ex[:, :], op=ALU.add,
                        axis=mybir.AxisListType.X)
                    rec_t = clp.tile([128, 1], f32, tag="rc")
                    nc.vector.reciprocal(rec_t[:, :], ssum[:, :])
                    sm = clp.tile([128, NCLS], f32, tag="sm")
                    nc.vector.tensor_scalar_mul(sm[:, :], ex[:, :], rec_t[:, :])
                    nc.gpsimd.dma_start(out=out_d[sl, :], in_=sm[:, :])

    return nc


# ---------------------------------------------------------------------------
def _prep_host(inputs):
    """Shard + pre-arrange all device inputs. Returns (in_maps, mask_entries,
    has_clsb)."""
    ids = np.asarray(inputs["ids"])
    emb = np.asarray(inputs["emb_table"], dtype=np.float32)

    def gate2(wk):
        w = np.array(wk, dtype=np.float32, copy=True)
        w[:, 2 * UNITS:3 * UNITS] *= 2.0
        return w

    def pad_k(w, kpad):
        out = np.zeros((kpad, G4), np.float32)
        out[:w.shape[0]] = w
        return out

    # ---- weight wall [128, CTOT] ----
    wall = np.zeros((128, CTOT), np.float32)
    w0 = np.stack([pad_k(gate2(inputs["fw0_k"]), 384),
                   pad_k(gate2(inputs["bw0_k"]), 384)])      # [2,384,512]
    wall[:, OFF_W0:OFF_R0] = (
        w0.reshape(2, 3, 128, G4).transpose(2, 0, 1, 3).reshape(128, 6 * G4))
    r0 = np.stack([gate2(inputs["fw0_r"]), gate2(inputs["bw0_r"])])
    wall[:, OFF_R0:OFF_W1] = r0.transpose(1, 0, 2).reshape(128, 2 * G4)
    w1 = np.stack([gate2(inputs["fw1_k"]), gate2(inputs["bw1_k"])])  # [2,256,512]
    wall[:, OFF_W1:OFF_R1] = (
        w1.reshape(2, 2, 128, G4).transpose(2, 0, 1, 3).reshape(128, 4 * G4))
    r1 = np.stack([gate2(inputs["fw1_r"]), gate2(inputs["bw1_r"])])
    wall[:, OFF_R1:OFF_CLS] = r1.transpose(1, 0, 2).reshape(128, 2 * G4)
    clsw = np.asarray(inputs["cls_w"], np.float32).reshape(2, 128, NCLS)
    wall[:, OFF_CLS:OFF_ID] = clsw.transpose(1, 0, 2).reshape(128, 2 * NCLS)
    wall[:, OFF_ID:CTOT] = np.eye(128, dtype=np.float32)
    wall_bf = wall.astype(ml_dtypes.bfloat16)

    def bias_tile(bf, bb_):
        out = np.zeros((128, 8), np.float32)
        for d, b in enumerate((bf, bb_)):
            b = np.array(b, dtype=np.float32, copy=True)
            b[2 * UNITS:3 * UNITS] *= 2.0
            out[:, 4 * d:4 * d + 4] = b.reshape(4, 128).T
        return out

    b0 = bias_tile(inputs["fw0_b"], inputs["bw0_b"])
    b1 = bias_tile(inputs["fw1_b"], inputs["bw1_b"])
    clsb_np = np.asarray(inputs["cls_b"], np.float32)
    has_clsb = bool(np.any(clsb_np != 0))

    # ---- embeddings: host gather + transpose + int8 quant ----
    x_all = emb[ids]                                        # [B, T, 300] f32
    sf = np.abs(x_all).reshape(-1, EMB).max(0) / 127.0      # per-feature scale
    sf = np.where(sf == 0, 1.0, sf)
    xs = np.ones((128, 3), np.float32)
    for k in range(3):
        nk = min(128, EMB - 128 * k)
        xs[:nk, k] = sf[128 * k:128 * k + nk]

    mask_entry_set = set()
    per_core = []
    for c in range(NCORES):
        ids_c = ids[BL * c:BL * (c + 1)]                    # [BL, T]
        x_c = x_all[BL * c:BL * (c + 1)]                    # [BL, T, 300]
        xt_c = x_c.transpose(2, 1, 0).reshape(EMB, NTOK)    # col j = 4t+e
        xq_c = np.clip(np.rint(xt_c / sf[:, None]), -127, 127).astype(np.int8)
        mask_c = (ids_c != 0)
        for e, t in zip(*np.nonzero(~mask_c)):
            mask_entry_set.add((0, int(t)))          # fwd step s = t
            mask_entry_set.add((1, int(511 - t)))    # bwd step s = 511 - t
        per_core.append((xq_c, mask_c))

    mask_entries = tuple(sorted(mask_entry_set))
    nmask = max(1, len(mask_entries))
    NB = 16 + 4 * nmask + (NCLS if has_clsb else 0)

    in_maps = []
    for c in range(NCORES):
        xq_c, mask_c = per_core[c]
        bbt = np.zeros((128, NB), np.float32)
        bbt[:, 0:8] = b0
        bbt[:, 8:16] = b1
        bbt[:, 16:16 + 4 * nmask] = 1.0
        for mi, (d, s) in enumerate(mask_entries):
            t = s if d == 0 else 511 - s
            bbt[:, 16 + 4 * mi:16 + 4 * mi + 4] = (
                mask_c[:, t].astype(np.float32)[None, :])
        if has_clsb:
            bbt[:, NB - NCLS:NB] = clsb_np[None, :]
        wsh_c = wall_bf[SHROWS * c:SHROWS * (c + 1), :].reshape(1, WSH)
        m = dict(xq=xq_c, xs=xs, wsh=wsh_c, bb=bbt)
        in_maps.append(m)
    return in_maps, mask_entries, has_clsb


# ---------------------------------------------------------------------------
def _make_fn(nc):
    """jit'd SPMD executor for the program. The donated output zero-buffers
    are created ON DEVICE once (device_put) and reused across calls — the
    kernel overwrites every output element, so their content is irrelevant
    after the first write. Returns (fn, in_names, dzeros) where dzeros are
    the device-resident trailing args."""
    import jax
    import concourse.mybir as mybir
    from concourse import bass2jax
    from jax.sharding import Mesh, PartitionSpec, NamedSharding
    from jax.experimental.shard_map import shard_map

    bass2jax.install_neuronx_cc_hook()
    partition_name = (nc.partition_id_tensor.name
                      if nc.partition_id_tensor else None)
    in_names, out_names, out_avals = [], [], []
    for alloc in nc.m.functions[0].allocations:
        if not isinstance(alloc, mybir.MemoryLocationSet):
            continue
        name = alloc.memorylocations[0].name
        if alloc.kind == "ExternalInput":
            if name != partition_name:
                in_names.append(name)
        elif alloc.kind == "ExternalOutput":
            shape = tuple(alloc.tensor_shape)
            dtype = mybir.dt.np(alloc.dtype)
            out_names.append(name)
            out_avals.append(jax.core.ShapedArray(shape, dtype))
    n_params = len(in_names)
    n_outs = len(out_avals)
    all_in_names = list(in_names) + list(out_names)
    if partition_name is not None:
        all_in_names.append(partition_name)

    def _body(*args):
        operands = list(args)
        if partition_name is not None:
            operands.append(bass2jax.partition_id_tensor())
        return tuple(bass2jax._bass_exec_p.bind(
            *operands, out_avals=tuple(out_avals), in_names=tuple(all_in_names),
            out_names=tuple(out_names), lowering_input_output_aliases=(),
            sim_require_finite=True, sim_require_nnan=True, nc=nc))

    devices = jax.devices()[:NCORES]
    mesh = Mesh(np.asarray(devices), ("core",))
    fn = jax.jit(shard_map(_body, mesh=mesh,
                           in_specs=(PartitionSpec("core"),) * (n_params + n_outs),
                           out_specs=(PartitionSpec("core"),) * n_outs,
                           check_rep=False), keep_unused=True)
    shard = NamedSharding(mesh, PartitionSpec("core"))
    dzeros = [jax.device_put(
        np.zeros((av.shape[0] * NCORES,) + tuple(av.shape[1:]), av.dtype), shard)
        for av in out_avals]
    jax.block_until_ready(dzeros)
    return fn, in_names, dzeros


def _get_fn(in_maps, mask_entries, has_clsb):
    key = (mask_entries, has_clsb)
    if key not in _prog_cache:
        _prog_cache[key] = _build_program(mask_entries, has_clsb)
    nc = _prog_cache[key]
    if key not in _fn_cache:
        _fn_cache[key] = _make_fn(nc)
    return _fn_cache[key]


def make_args(in_maps, in_names):
    return [np.concatenate([np.asarray(in_maps[c][nm]) for c in range(NCORES)],
                           axis=0) for nm in in_names]


# ---------------------------------------------------------------------------
def kernel(**inputs):
    in_maps, mask_entries, has_clsb = _prep_host(inputs)
    fn, in_names, dzeros = _get_fn(in_maps, mask_entries, has_clsb)
    args = make_args(in_maps, in_names) + dzeros
    outs = fn(*args)
    oc = np.asarray(outs[0]).reshape(NCORES, T, BL, NCLS)
    return np.ascontiguousarray(
        oc.transpose(0, 2, 1, 3).reshape(B, T, NCLS)).astype(np.float32)


# revision 14
# speedup vs baseline: 5.9415x; 1.4906x over previous
"""Trainium2 Bass kernel for a 2-layer BiLSTM text tagger.

Model (see reference): embedding gather -> BiLSTM(128) -> BiLSTM(128) with
residual -> dense(279) -> softmax. mask_zero=True semantics (state + output
carry-through at masked steps).

Sharding: data-parallel over batch, 4 examples per core on 8 cores.

The per-call cost in this environment is dominated by host->device transfer
over the axon relay (~50-90 MB/s, ~80 ms dispatch floor), so the I/O layout
is optimized for minimum bytes on the wire:
  - the embedding gather + transpose happens on HOST; the device receives
    xq [300, 2048] int8 (per-feature-scaled) per core (~0.6 MB/core),
    dequantized to bf16 on device with one DVE op per 128-feature chunk.
  - all weights (layer0/1 kernels+recurrent, classifier, identity) are
    packed into ONE [128, CTOT] bf16 "wall"; each core receives 1/8 of it
    (16 rows) and an on-device DRAM AllGather reconstructs the full wall
    (2 MB total over the wire instead of 16 MB replicated).
  - biases/masks ride in one small f32 [128, NB] tile.
  - donated output zero-buffers are created on device (jnp.zeros inside the
    jitted body), not transferred.

Device layout (per core, feature/gate dim on partitions, batch in free dim):
  XT[k]  [128, 2048] bf16  - embeddings, feature = 128k+p, col j = 4t+e
  Zb     [128, 16384] bf16 - input projections in PSUM-bank order:
                             col = 32s + 16d + 4c + e (s step, d dir, c gate
                             chunk i/f/g/o, e example). g-chunk pre-scaled by
                             2 so one Sigmoid computes i,f,o sigmoids and
                             sigma(2 z_g) (tanh via 2*sig(2x)-1).
  H*     [128, 2048] bf16  - hidden states, col = 4t + e
  Recurrence step: one identity-matmul injects 16 steps of Z into a PSUM
  bank (start=True), then per step 8 accumulating matmuls add h @ Wr per
  (dir, gate-chunk); Sigmoid reads the 32-col slice; DVE computes the cell
  update with a fused scalar_tensor_tensor for the tanh fix-up.
"""

import json

import ml_dtypes
import numpy as np

# ---------------------------------------------------------------------------
# problem constants (hardcoded per the contract)
B, T = 32, 512
EMB, UNITS, NCLS = 300, 128, 279
VOCAB = 100000
NCORES = 8
BL = B // NCORES          # 4 examples / core
NTOK = BL * T             # 2048 tokens / core
G4 = 4 * UNITS            # 512
NTILE = NTOK // 128       # 16 token tiles

# wall (packed weights) column layout. Stored as fp8e3 (e3m4) scaled by
# WSCALE=16 so the small LSTM weights sit in e3m4's normal range; the /16 is
# folded into the sigmoid/exp activation `scale` (zb and the PSUM gate banks
# carry 16x values throughout). The identity section stays 1.0 (it only
# passes 16x-scaled zb through).
OFF_W0 = 0                # [128, 2, 3, 512]  (k=2 chunk rows >=44 are zero)
OFF_R0 = OFF_W0 + 2 * 3 * G4          # 3072
OFF_W1 = OFF_R0 + 2 * G4              # 4096
OFF_R1 = OFF_W1 + 2 * 2 * G4          # 6144
OFF_CLS = OFF_R1 + 2 * G4             # 7168, [128, 2, 279]
OFF_ID = OFF_CLS + 2 * NCLS           # 7726, [128, 128] identity
CTOT = OFF_ID + 128                   # 7854
WTOT = 128 * CTOT                     # 1,005,312 elems
WSH = WTOT // NCORES                  # 125,664 elems (= 16 rows)
SHROWS = 128 // NCORES                # 16 rows per shard
WSCALE = 16.0
WINV = 1.0 / WSCALE
XPK = 3 * (NTOK // 4)                 # 1536 packed int6 cols (3 B / 4 vals)

_prog_cache = {}
_fn_cache = {}


# ---------------------------------------------------------------------------
def _apply_bir_wait_split(bass_mod):
    """This container's walrus rejects >1 sync-wait per instruction. Split
    extras onto inserted EventSemaphore instructions (same engine, in order).
    """
    if getattr(bass_mod.Bass, "_wait_split_applied", False):
        return
    orig = bass_mod.Bass.to_json_bytes
    ctr = [0]

    def fix_list(lst):
        out, changed = [], False
        for ins in lst:
            si = ins.get("sync_info") if isinstance(ins, dict) else None
            if not si:
                out.append(ins)
                continue
            waits = si.get("on_wait") or []
            upds = si.get("on_update") or []
            if len(waits) > 1:
                for w in waits[1:]:
                    ctr[0] += 1
                    out.append({
                        "debug": ins.get("debug", 0), "engine": ins["engine"],
                        "ins": [], "name": f"I-waitfix-{ctr[0]}",
                        "opcode": "EventSemaphore", "outs": [],
                        "sync_info": {"on_update": [], "on_wait": [w]},
                    })
                si["on_wait"] = waits[:1]
                changed = True
            out.append(ins)
            if len(upds) > 1:
                for u in upds[1:]:
                    ctr[0] += 1
                    out.append({
                        "debug": ins.get("debug", 0), "engine": ins["engine"],
                        "ins": [], "name": f"I-updfix-{ctr[0]}",
                        "opcode": "EventSemaphore", "outs": [],
                        "sync_info": {"on_update": [u], "on_wait": []},
                    })
                si["on_update"] = upds[:1]
                changed = True
        return out, changed

    def walk(o):
        if isinstance(o, dict):
            for k, v in o.items():
                if (isinstance(v, list) and v
                        and all(isinstance(e, dict) and "opcode" in e for e in v)):
                    fixed, changed = fix_list(v)
                    if changed:
                        o[k] = fixed
                    for e in o[k]:
                        walk(e)
                else:
                    walk(v)
        elif isinstance(o, list):
            for v in o:
                walk(v)

    def to_json_bytes_fixed(self):
        d = json.loads(orig(self))
        walk(d)
        return json.dumps(d).encode()

    bass_mod.Bass.to_json_bytes = to_json_bytes_fixed
    bass_mod.Bass._wait_split_applied = True


# ---------------------------------------------------------------------------
def _build_program(mask_entries, has_clsb):
    """Build the Bass program (shared by all 8 cores).

    mask_entries: sorted tuple of (d, s) recurrence slots that need the
    data-driven carry-through lerp (d: 0 fwd / 1 bwd, s: step index).
    """
    import concourse.bass as bass
    import concourse.mybir as mybir
    import concourse.tile as tile

    _apply_bir_wait_split(bass)

    bf16 = mybir.dt.bfloat16
    f32 = mybir.dt.float32
    u8 = mybir.dt.uint8
    f8 = mybir.dt.float8e3
    AF = mybir.ActivationFunctionType
    ALU = mybir.AluOpType

    nc = bass.Bass(num_devices=NCORES)

    nmask = max(1, len(mask_entries))
    NB = 16 + 4 * nmask + (NCLS if has_clsb else 0)

    # ---- DRAM I/O ----
    xq_d = nc.dram_tensor("xq", [EMB, XPK], u8, kind="ExternalInput")
    xs_d = nc.dram_tensor("xs", [128, 3], f32, kind="ExternalInput")
    wsh_d = nc.dram_tensor("wsh", [1, WSH], f8, kind="ExternalInput")
    bb_d = nc.dram_tensor("bb", [128, NB], f32, kind="ExternalInput")
    out_d = nc.dram_tensor("out", [NTOK, NCLS], f32, kind="ExternalOutput")

    mask_idx = {ds: i for i, ds in enumerate(mask_entries)}

    with tile.TileContext(nc) as tc:
        with (
            tc.tile_pool(name="const", bufs=1) as cpool,
            tc.tile_pool(name="big", bufs=1) as bigpool,
            tc.tile_pool(name="state", bufs=1) as spool,
            tc.tile_pool(name="dram", bufs=1, space="DRAM") as dram,
        ):
            # ---- weight wall: fp8 shard -> DRAM AllGather -> SBUF -> bf16 ----
            inb = dram.tile([1, WSH], f8)
            outb = dram.tile([128, CTOT], f8)
            nc.gpsimd.dma_start(out=inb[:, :], in_=wsh_d[:, :])
            nc.gpsimd.collective_compute(
                "AllGather", mybir.AluOpType.bypass,
                replica_groups=[list(range(NCORES))],
                ins=[inb[:, :].opt()], outs=[outb[:, :].opt()],
            )
            wall8 = bigpool.tile([128, CTOT], f8)
            nc.gpsimd.dma_start(out=wall8[:, :], in_=outb[:, :])
            wall = bigpool.tile([128, CTOT], bf16)
            nc.vector.tensor_copy(wall[:, :], wall8[:, :])

            # ---- small constants ----
            xs_sb = cpool.tile([128, 3], f32)
            nc.gpsimd.dma_start(out=xs_sb[:, :], in_=xs_d[:, :])
            bb = cpool.tile([128, NB], f32)
            nc.gpsimd.dma_start(out=bb[:, :], in_=bb_d[:, :])

            # ---- embeddings: packed int6 -> bf16 dequant ----
            # 4 values (examples e=0..3 of one t) in 3 bytes: lanes a,b,c in
            # low-6 of bytes 0..2; lane d's 6 bits in the top-2 of each byte.
            xt = []
            NG = NTOK // 4     # 512 groups
            with tc.tile_pool(name="xqp", bufs=1) as xqp:
                for k in range(3):
                    nk = min(128, EMB - 128 * k)
                    xqt = xqp.tile([128, XPK], u8, tag=f"xq{k}", name=f"xq{k}")
                    nc.gpsimd.dma_start(
                        out=xqt[:nk, :], in_=xq_d[128 * k:128 * k + nk, :])
                    xk = bigpool.tile([128, NTOK], bf16, tag=f"xt{k}",
                                      name=f"xt{k}")
                    if nk < 128:
                        nc.vector.memset(xk[:, :], 0.0)

                    xq_nk = xqt[:nk, :]
                    xk_nk = xk[:nk, :]

                    def plane(j):
                        return bass.AP(tensor=xq_nk.tensor,
                                       offset=xq_nk.offset + j,
                                       ap=[xq_nk.ap[0]] + [[3, NG]])

                    def outp(e):
                        return bass.AP(tensor=xk_nk.tensor,
                                       offset=xk_nk.offset + e,
                                       ap=[xk_nk.ap[0]] + [[4, NG]])

                    sc = xs_sb[:nk, k:k + 1]
                    tmp = xqp.tile([128, NG], u8, tag="tmp")
                    for e in range(3):
                        nc.vector.tensor_scalar(
                            out=tmp[:nk, :], in0=plane(e), scalar1=63,
                            scalar2=None, op0=ALU.bitwise_and)
                        nc.vector.tensor_scalar(
                            out=outp(e), in0=tmp[:nk, :], scalar1=32.0,
                            scalar2=sc, op0=ALU.subtract, op1=ALU.mult)
                    t0 = xqp.tile([128, NG], u8, tag="t0")
                    nc.vector.tensor_scalar(
                        out=t0[:nk, :], in0=plane(0), scalar1=6,
                        scalar2=None, op0=ALU.logical_shift_right)
                    t1 = xqp.tile([128, NG], u8, tag="t1")
                    nc.vector.tensor_scalar(
                        out=t1[:nk, :], in0=plane(1), scalar1=6, scalar2=2,
                        op0=ALU.logical_shift_right, op1=ALU.logical_shift_left)
                    nc.vector.tensor_tensor(
                        out=t0[:nk, :], in0=t0[:nk, :], in1=t1[:nk, :],
                        op=ALU.bitwise_or)
                    nc.vector.tensor_scalar(
                        out=t1[:nk, :], in0=plane(2), scalar1=6, scalar2=4,
                        op0=ALU.logical_shift_right, op1=ALU.logical_shift_left)
                    nc.vector.tensor_tensor(
                        out=t0[:nk, :], in0=t0[:nk, :], in1=t1[:nk, :],
                        op=ALU.bitwise_or)
                    nc.vector.tensor_scalar(
                        out=outp(3), in0=t0[:nk, :], scalar1=32.0,
                        scalar2=sc, op0=ALU.subtract, op1=ALU.mult)
                    xt.append(xk)

            # ---- big persistent buffers ----
            zb = bigpool.tile([128, 32 * T], bf16)
            h0f = bigpool.tile([128, NTOK], bf16)
            h0b = bigpool.tile([128, NTOK], bf16)
            h1f = bigpool.tile([128, NTOK], bf16)
            h1b = bigpool.tile([128, NTOK], bf16)

            hz = spool.tile([128, 8], bf16)
            nc.vector.memset(hz[:, :], 0.0)

            def strided(tileap, offset, dims):
                return bass.AP(tensor=tileap.tensor, offset=tileap.offset + offset,
                               ap=[tileap.ap[0]] + dims)

            ident = wall[:, OFF_ID:OFF_ID + 128]

            # ================= shared phase helpers =================
            def projection(layer):
                """Compute Zb for `layer` from its inputs (XT or H0)."""
                nk = 3 if layer == 0 else 2
                woff = OFF_W0 if layer == 0 else OFF_W1
                boff = 0 if layer == 0 else 8
                with tc.tile_pool(name=f"pj{layer}", bufs=4, space="PSUM") as pjp:
                    for d in range(2):
                        for c in range(4):
                            for nb in range(4):
                                ps = pjp.tile([128, 512], f32, tag="pj")
                                s0 = 128 * nb
                                for k in range(nk):
                                    if layer == 0:
                                        src = xt[k][:, :]
                                    else:
                                        src = (h0f if k == 0 else h0b)[:, :]
                                    if d == 0:
                                        rhs = strided(src, 4 * s0,
                                                      [[4, 128], [1, 4]])
                                    else:
                                        rhs = strided(src, 4 * (511 - s0),
                                                      [[-4, 128], [1, 4]])
                                    wcol = woff + (d * nk + k) * G4 + c * 128
                                    nc.tensor.matmul(
                                        ps[:, :],
                                        wall[:, wcol:wcol + 128],
                                        rhs, start=(k == 0), stop=(k == nk - 1))
                                dst = strided(zb[:, :], 32 * s0 + 16 * d + 4 * c,
                                              [[32, 128], [1, 4]])
                                nc.scalar.activation(
                                    dst, ps[:, :], AF.Identity,
                                    bias=bb[:, boff + 4 * d + c:boff + 4 * d + c + 1],
                                    scale=1.0)

            def recurrence(layer):
                roff = OFF_R0 if layer == 0 else OFF_R1
                Hf = h0f if layer == 0 else h1f
                Hb = h0b if layer == 0 else h1b
                with (
                    tc.tile_pool(name=f"rc{layer}", bufs=6, space="PSUM") as rcp,
                    tc.tile_pool(name=f"gt{layer}", bufs=8) as gtp,
                    tc.tile_pool(name=f"tm{layer}", bufs=8) as tmp,
                ):
                    c_state = spool.tile([128, 8], f32, tag=f"c{layer}")
                    nc.vector.memset(c_state[:, :], 0.0)
                    ps = None
                    prev_ht = None
                    for s in range(T):
                        sb = s % 16
                        if sb == 0:
                            ps = rcp.tile([128, 512], f32, tag="bank")
                            nc.tensor.matmul(
                                ps[:, :], ident,
                                zb[:, 512 * (s // 16):512 * (s // 16) + 512],
                                start=True, stop=False, skip_group_check=True)
                        for d in range(2):
                            if s == 0:
                                hprev = hz[:, 4 * d:4 * d + 4]
                            elif prev_ht is not None:
                                hprev = prev_ht[:, 4 * d:4 * d + 4]
                            elif d == 0:
                                hprev = Hf[:, 4 * (s - 1):4 * (s - 1) + 4]
                            else:
                                hprev = Hb[:, 4 * (512 - s):4 * (512 - s) + 4]
                            for c in range(4):
                                rcol = roff + d * G4 + c * 128
                                nc.tensor.matmul(
                                    ps[:, 32 * sb + 16 * d + 4 * c:
                                       32 * sb + 16 * d + 4 * c + 4],
                                    wall[:, rcol:rcol + 128],
                                    hprev, start=False, stop=False,
                                    skip_group_check=True)
                        sg = gtp.tile([128, 32], f32, tag="sg")
                        nc.scalar.activation(
                            sg[:, :], ps[:, 32 * sb:32 * sb + 32], AF.Sigmoid,
                            scale=WINV)
                        sga = sg[:, :]
                        i_ap = strided(sga, 0, [[16, 2], [1, 4]])
                        f_ap = strided(sga, 4, [[16, 2], [1, 4]])
                        g_ap = strided(sga, 8, [[16, 2], [1, 4]])
                        # i*(2g'-1) = 2*i*(g'-0.5): one fused op; the *2 folds
                        # into the final accumulate.
                        w_t = tmp.tile([128, 8], f32, tag="w")
                        nc.vector.scalar_tensor_tensor(
                            out=w_t[:, :], in0=g_ap, scalar=0.5, in1=i_ap,
                            op0=ALU.subtract, op1=ALU.mult)
                        v = tmp.tile([128, 8], f32, tag="v")
                        nc.vector.tensor_tensor(
                            out=v[:, :], in0=f_ap, in1=c_state[:, :], op=ALU.mult)
                        masked = [d for d in range(2) if (d, s) in mask_idx]
                        if not masked:
                            nc.vector.scalar_tensor_tensor(
                                out=c_state[:, :], in0=w_t[:, :], scalar=2.0,
                                in1=v[:, :], op0=ALU.mult, op1=ALU.add)
                            th = tmp.tile([128, 8], f32, tag="th")
                            nc.scalar.activation(th[:, :], c_state[:, :], AF.Tanh)
                            o_ap = strided(sga, 12, [[16, 2], [1, 4]])
                            ht = tmp.tile([128, 8], bf16, tag="ht")
                            nc.vector.tensor_tensor(
                                out=ht[:, :], in0=o_ap, in1=th[:, :],
                                op=ALU.mult)
                            nc.vector.tensor_copy(
                                Hf[:, 4 * s:4 * s + 4], ht[:, 0:4])
                            nc.vector.tensor_copy(
                                Hb[:, 4 * (511 - s):4 * (511 - s) + 4],
                                ht[:, 4:8])
                            prev_ht = ht
                        else:
                            cc = tmp.tile([128, 8], f32, tag="cc")
                            nc.vector.scalar_tensor_tensor(
                                out=cc[:, :], in0=w_t[:, :], scalar=2.0,
                                in1=v[:, :], op0=ALU.mult, op1=ALU.add)
                            # c lerp: cc_d = c_old + m*(cc_d - c_old)
                            for d in masked:
                                mi = mask_idx[(d, s)]
                                mcol = bb[:, 16 + 4 * mi:16 + 4 * mi + 4]
                                dd = tmp.tile([128, 4], f32, tag="dd")
                                nc.vector.tensor_tensor(
                                    out=dd[:, :], in0=cc[:, 4 * d:4 * d + 4],
                                    in1=c_state[:, 4 * d:4 * d + 4], op=ALU.subtract)
                                nc.vector.tensor_tensor(
                                    out=dd[:, :], in0=dd[:, :], in1=mcol, op=ALU.mult)
                                nc.vector.tensor_tensor(
                                    out=cc[:, 4 * d:4 * d + 4], in0=dd[:, :],
                                    in1=c_state[:, 4 * d:4 * d + 4], op=ALU.add)
                            nc.vector.tensor_copy(c_state[:, :], cc[:, :])
                            th = tmp.tile([128, 8], f32, tag="th")
                            nc.scalar.activation(th[:, :], c_state[:, :], AF.Tanh)
                            for d in range(2):
                                o_sl = sg[:, 16 * d + 12:16 * d + 16]
                                th_sl = th[:, 4 * d:4 * d + 4]
                                dst = (Hf[:, 4 * s:4 * s + 4] if d == 0 else
                                       Hb[:, 4 * (511 - s):4 * (511 - s) + 4])
                                if d in masked:
                                    mi = mask_idx[(d, s)]
                                    mcol = bb[:, 16 + 4 * mi:16 + 4 * mi + 4]
                                    if s == 0:
                                        hp = hz[:, 4 * d:4 * d + 4]
                                    elif d == 0:
                                        hp = Hf[:, 4 * (s - 1):4 * (s - 1) + 4]
                                    else:
                                        hp = Hb[:, 4 * (512 - s):4 * (512 - s) + 4]
                                    hn = tmp.tile([128, 4], f32, tag="hn")
                                    nc.vector.tensor_tensor(
                                        out=hn[:, :], in0=o_sl, in1=th_sl,
                                        op=ALU.mult)
                                    nc.vector.tensor_tensor(
                                        out=hn[:, :], in0=hn[:, :], in1=hp,
                                        op=ALU.subtract)
                                    nc.vector.tensor_tensor(
                                        out=hn[:, :], in0=hn[:, :], in1=mcol,
                                        op=ALU.mult)
                                    nc.vector.tensor_tensor(
                                        out=dst, in0=hn[:, :], in1=hp, op=ALU.add)
                                else:
                                    nc.vector.tensor_tensor(
                                        out=dst, in0=o_sl, in1=th_sl, op=ALU.mult)
                            prev_ht = None

            # ================= run the phases =================
            projection(0)
            recurrence(0)
            projection(1)
            recurrence(1)

            # ================= classifier + softmax =================
            with (
                tc.tile_pool(name="cls", bufs=4) as clp,
                tc.tile_pool(name="clps", bufs=4, space="PSUM") as clps,
            ):
                for tt in range(NTILE):
                    sl = slice(128 * tt, 128 * (tt + 1))
                    i0 = clp.tile([128, 128], bf16, tag="i0")
                    nc.vector.tensor_tensor(
                        out=i0[:, :], in0=h0f[:, sl], in1=h1f[:, sl], op=ALU.add)
                    i1 = clp.tile([128, 128], bf16, tag="i1")
                    nc.vector.tensor_tensor(
                        out=i1[:, :], in0=h0b[:, sl], in1=h1b[:, sl], op=ALU.add)
                    pc = clps.tile([128, NCLS], f32, tag="pc")
                    nc.tensor.matmul(pc[:, :], i0[:, :],
                                     wall[:, OFF_CLS:OFF_CLS + NCLS],
                                     start=True, stop=False)
                    nc.tensor.matmul(pc[:, :], i1[:, :],
                                     wall[:, OFF_CLS + NCLS:OFF_CLS + 2 * NCLS],
                                     start=False, stop=True)
                    ex = clp.tile([128, NCLS], f32, tag="ex")
                    if has_clsb:
                        # bb holds 16*clsb, so Exp(scale/16) yields
                        # exp(logits + clsb).
                        nc.vector.tensor_tensor(
                            out=ex[:, :], in0=pc[:, :],
                            in1=bb[:, NB - NCLS:NB], op=ALU.add)
                        nc.scalar.activation(ex[:, :], ex[:, :], AF.Exp,
                                             scale=WINV)
                    else:
                        nc.scalar.activation(ex[:, :], pc[:, :], AF.Exp,
                                             scale=WINV)
                    ssum = clp.tile([128, 1], f32, tag="ss")
                    nc.vector.tensor_reduce(
                        out=ssum[:, :], in_=ex[:, :], op=ALU.add,
                        axis=mybir.AxisListType.X)
                    rec_t = clp.tile([128, 1], f32, tag="rc")
                    nc.vector.reciprocal(rec_t[:, :], ssum[:, :])
                    sm = clp.tile([128, NCLS], f32, tag="sm")
                    nc.vector.tensor_scalar_mul(sm[:, :], ex[:, :], rec_t[:, :])
                    nc.gpsimd.dma_start(out=out_d[sl, :], in_=sm[:, :])

    return nc


# ---------------------------------------------------------------------------
def _prep_host(inputs):
    """Shard + pre-arrange all device inputs. Returns (in_maps, mask_entries,
    has_clsb)."""
    ids = np.asarray(inputs["ids"])
    emb = np.asarray(inputs["emb_table"], dtype=np.float32)

    def gate2(wk):
        w = np.array(wk, dtype=np.float32, copy=True)
        w[:, 2 * UNITS:3 * UNITS] *= 2.0
        return w

    def pad_k(w, kpad):
        out = np.zeros((kpad, G4), np.float32)
        out[:w.shape[0]] = w
        return out

    # ---- weight wall [128, CTOT], stored fp8e3 scaled by WSCALE ----
    wall = np.zeros((128, CTOT), np.float32)
    w0 = np.stack([pad_k(gate2(inputs["fw0_k"]), 384),
                   pad_k(gate2(inputs["bw0_k"]), 384)])      # [2,384,512]
    wall[:, OFF_W0:OFF_R0] = (
        w0.reshape(2, 3, 128, G4).transpose(2, 0, 1, 3).reshape(128, 6 * G4))
    r0 = np.stack([gate2(inputs["fw0_r"]), gate2(inputs["bw0_r"])])
    wall[:, OFF_R0:OFF_W1] = r0.transpose(1, 0, 2).reshape(128, 2 * G4)
    w1 = np.stack([gate2(inputs["fw1_k"]), gate2(inputs["bw1_k"])])  # [2,256,512]
    wall[:, OFF_W1:OFF_R1] = (
        w1.reshape(2, 2, 128, G4).transpose(2, 0, 1, 3).reshape(128, 4 * G4))
    r1 = np.stack([gate2(inputs["fw1_r"]), gate2(inputs["bw1_r"])])
    wall[:, OFF_R1:OFF_CLS] = r1.transpose(1, 0, 2).reshape(128, 2 * G4)
    clsw = np.asarray(inputs["cls_w"], np.float32).reshape(2, 128, NCLS)
    wall[:, OFF_CLS:OFF_ID] = clsw.transpose(1, 0, 2).reshape(128, 2 * NCLS)
    wall *= WSCALE
    wall[:, OFF_ID:CTOT] = np.eye(128, dtype=np.float32)  # identity stays 1.0
    wall_bf = wall.astype(ml_dtypes.float8_e3m4)

    def bias_tile(bf, bb_):
        out = np.zeros((128, 8), np.float32)
        for d, b in enumerate((bf, bb_)):
            b = np.array(b, dtype=np.float32, copy=True)
            b[2 * UNITS:3 * UNITS] *= 2.0
            out[:, 4 * d:4 * d + 4] = b.reshape(4, 128).T
        return out

    b0 = bias_tile(inputs["fw0_b"], inputs["bw0_b"])
    b1 = bias_tile(inputs["fw1_b"], inputs["bw1_b"])
    clsb_np = np.asarray(inputs["cls_b"], np.float32)
    has_clsb = bool(np.any(clsb_np != 0))

    # ---- embeddings: host gather + transpose + packed int6 quant ----
    x_all = emb[ids]                                        # [B, T, 300] f32
    sf = np.abs(x_all).reshape(-1, EMB).max(0) / 31.0       # per-feature scale
    sf = np.where(sf == 0, 1.0, sf)
    xs = np.ones((128, 3), np.float32)
    for k in range(3):
        nk = min(128, EMB - 128 * k)
        xs[:nk, k] = sf[128 * k:128 * k + nk]

    mask_entry_set = set()
    per_core = []
    for c in range(NCORES):
        ids_c = ids[BL * c:BL * (c + 1)]                    # [BL, T]
        x_c = x_all[BL * c:BL * (c + 1)]                    # [BL, T, 300]
        xt_c = x_c.transpose(2, 1, 0).reshape(EMB, NTOK)    # col j = 4t+e
        u = (np.clip(np.rint(xt_c / sf[:, None]), -31, 31) + 32).astype(
            np.uint8).reshape(EMB, NTOK // 4, 4)
        p0 = u[..., 0] | ((u[..., 3] & 3) << 6)
        p1 = u[..., 1] | (((u[..., 3] >> 2) & 3) << 6)
        p2 = u[..., 2] | ((u[..., 3] >> 4) << 6)
        xq_c = np.stack([p0, p1, p2], axis=-1).reshape(EMB, XPK)
        mask_c = (ids_c != 0)
        for e, t in zip(*np.nonzero(~mask_c)):
            mask_entry_set.add((0, int(t)))          # fwd step s = t
            mask_entry_set.add((1, int(511 - t)))    # bwd step s = 511 - t
        per_core.append((xq_c, mask_c))

    mask_entries = tuple(sorted(mask_entry_set))
    nmask = max(1, len(mask_entries))
    NB = 16 + 4 * nmask + (NCLS if has_clsb else 0)

    in_maps = []
    for c in range(NCORES):
        xq_c, mask_c = per_core[c]
        bbt = np.zeros((128, NB), np.float32)
        # zb carries WSCALE*z, so the projection biases ride scaled too
        bbt[:, 0:8] = b0 * WSCALE
        bbt[:, 8:16] = b1 * WSCALE
        bbt[:, 16:16 + 4 * nmask] = 1.0
        for mi, (d, s) in enumerate(mask_entries):
            t = s if d == 0 else 511 - s
            bbt[:, 16 + 4 * mi:16 + 4 * mi + 4] = (
                mask_c[:, t].astype(np.float32)[None, :])
        if has_clsb:
            bbt[:, NB - NCLS:NB] = clsb_np[None, :] * WSCALE
        wsh_c = wall_bf[SHROWS * c:SHROWS * (c + 1), :].reshape(1, WSH)
        m = dict(xq=xq_c, xs=xs, wsh=wsh_c, bb=bbt)
        in_maps.append(m)
    return in_maps, mask_entries, has_clsb


# ---------------------------------------------------------------------------
def _make_fn(nc):
    """jit'd SPMD executor for the program. The donated output zero-buffers
    are created ON DEVICE once (device_put) and reused across calls — the
    kernel overwrites every output element, so their content is irrelevant
    after the first write. Returns (fn, in_names, dzeros) where dzeros are
    the device-resident trailing args."""
    import jax
    import concourse.mybir as mybir
    from concourse import bass2jax
    from jax.sharding import Mesh, PartitionSpec, NamedSharding
    from jax.experimental.shard_map import shard_map

    bass2jax.install_neuronx_cc_hook()
    partition_name = (nc.partition_id_tensor.name
                      if nc.partition_id_tensor else None)
    in_names, out_names, out_avals = [], [], []
    for alloc in nc.m.functions[0].allocations:
        if not isinstance(alloc, mybir.MemoryLocationSet):
            continue
        name = alloc.memorylocations[0].name
        if alloc.kind == "ExternalInput":
            if name != partition_name:
                in_names.append(name)
        elif alloc.kind == "ExternalOutput":
            shape = tuple(alloc.tensor_shape)
            dtype = mybir.dt.np(alloc.dtype)
            out_names.append(name)
            out_avals.append(jax.core.ShapedArray(shape, dtype))
    n_params = len(in_names)
    n_outs = len(out_avals)
    all_in_names = list(in_names) + list(out_names)
    if partition_name is not None:
        all_in_names.append(partition_name)

    def _body(*args):
        operands = list(args)
        if partition_name is not None:
            operands.append(bass2jax.partition_id_tensor())
        return tuple(bass2jax._bass_exec_p.bind(
            *operands, out_avals=tuple(out_avals), in_names=tuple(all_in_names),
            out_names=tuple(out_names), lowering_input_output_aliases=(),
            sim_require_finite=True, sim_require_nnan=True, nc=nc))

    devices = jax.devices()[:NCORES]
    mesh = Mesh(np.asarray(devices), ("core",))
    fn = jax.jit(shard_map(_body, mesh=mesh,
                           in_specs=(PartitionSpec("core"),) * (n_params + n_outs),
                           out_specs=(PartitionSpec("core"),) * n_outs,
                           check_rep=False), keep_unused=True)
    shard = NamedSharding(mesh, PartitionSpec("core"))
    dzeros = [jax.device_put(
        np.zeros((av.shape[0] * NCORES,) + tuple(av.shape[1:]), av.dtype), shard)
        for av in out_avals]
    jax.block_until_ready(dzeros)
    return fn, in_names, dzeros


def _get_fn(in_maps, mask_entries, has_clsb):
    key = (mask_entries, has_clsb)
    if key not in _prog_cache:
        _prog_cache[key] = _build_program(mask_entries, has_clsb)
    nc = _prog_cache[key]
    if key not in _fn_cache:
        _fn_cache[key] = _make_fn(nc)
    return _fn_cache[key]


def make_args(in_maps, in_names):
    return [np.concatenate([np.asarray(in_maps[c][nm]) for c in range(NCORES)],
                           axis=0) for nm in in_names]


# ---------------------------------------------------------------------------
def kernel(**inputs):
    in_maps, mask_entries, has_clsb = _prep_host(inputs)
    fn, in_names, dzeros = _get_fn(in_maps, mask_entries, has_clsb)
    args = make_args(in_maps, in_names) + dzeros
    outs = fn(*args)
    oc = np.asarray(outs[0]).reshape(NCORES, T, BL, NCLS)
    return np.ascontiguousarray(
        oc.transpose(0, 2, 1, 3).reshape(B, T, NCLS)).astype(np.float32)


# revision 25
# speedup vs baseline: 6.1987x; 1.0433x over previous
"""Trainium2 Bass kernel for a 2-layer BiLSTM text tagger.

Model (see reference): embedding gather -> BiLSTM(128) -> BiLSTM(128) with
residual -> dense(279) -> softmax. mask_zero=True semantics (state + output
carry-through at masked steps).

Sharding: data-parallel over batch, 4 examples per core on 8 cores.

The per-call cost in this environment is dominated by host->device transfer
over the axon relay (~50-90 MB/s, ~80 ms dispatch floor), so the I/O layout
is optimized for minimum bytes on the wire:
  - the embedding gather + transpose happens on HOST; the device receives
    xq [300, 2048] int8 (per-feature-scaled) per core (~0.6 MB/core),
    dequantized to bf16 on device with one DVE op per 128-feature chunk.
  - all weights (layer0/1 kernels+recurrent, classifier, identity) are
    packed into ONE [128, CTOT] bf16 "wall"; each core receives 1/8 of it
    (16 rows) and an on-device DRAM AllGather reconstructs the full wall
    (2 MB total over the wire instead of 16 MB replicated).
  - biases/masks ride in one small f32 [128, NB] tile.
  - donated output zero-buffers are created on device (jnp.zeros inside the
    jitted body), not transferred.

Device layout (per core, feature/gate dim on partitions, batch in free dim):
  XT[k]  [128, 2048] bf16  - embeddings, feature = 128k+p, col j = 4t+e
  Zb     [128, 16384] bf16 - input projections in PSUM-bank order:
                             col = 32s + 16d + 4c + e (s step, d dir, c gate
                             chunk i/f/g/o, e example). g-chunk pre-scaled by
                             2 so one Sigmoid computes i,f,o sigmoids and
                             sigma(2 z_g) (tanh via 2*sig(2x)-1).
  H*     [128, 2048] bf16  - hidden states, col = 4t + e
  Recurrence step: one identity-matmul injects 16 steps of Z into a PSUM
  bank (start=True), then per step 8 accumulating matmuls add h @ Wr per
  (dir, gate-chunk); Sigmoid reads the 32-col slice; DVE computes the cell
  update with a fused scalar_tensor_tensor for the tanh fix-up.
"""

import json

import ml_dtypes
import numpy as np

# ---------------------------------------------------------------------------
# problem constants (hardcoded per the contract)
B, T = 32, 512
EMB, UNITS, NCLS = 300, 128, 279
VOCAB = 100000
NCORES = 8
BL = B // NCORES          # 4 examples / core
NTOK = BL * T             # 2048 tokens / core
G4 = 4 * UNITS            # 512
NTILE = NTOK // 128       # 16 token tiles

# wall (packed weights) column layout. Stored as fp8e3 (e3m4) scaled by
# WSCALE=16 so the small LSTM weights sit in e3m4's normal range; the /16 is
# folded into the sigmoid/exp activation `scale` (zb and the PSUM gate banks
# carry 16x values throughout). The identity section stays 1.0 (it only
# passes 16x-scaled zb through).
OFF_W0 = 0                # [128, 2, 3, 512]  (k=2 chunk rows >=44 are zero)
OFF_R0 = OFF_W0 + 2 * 3 * G4          # 3072
OFF_W1 = OFF_R0 + 2 * G4              # 4096
OFF_R1 = OFF_W1 + 2 * 2 * G4          # 6144
OFF_CLS = OFF_R1 + 2 * G4             # 7168, [128, 2, 279]
OFF_ID = OFF_CLS + 2 * NCLS           # 7726, [128, 128] identity
CTOT = OFF_ID + 128                   # 7854
WTOT = 128 * CTOT                     # 1,005,312 elems
WSH = WTOT // NCORES                  # 125,664 elems (= 16 rows)
SHROWS = 128 // NCORES                # 16 rows per shard
WSCALE = 16.0
WINV = 1.0 / WSCALE
NG = NTOK // 8                        # 256 int5 groups (8 values / 5 bytes)
XPK = 5 * NG                          # 1280 packed cols
XALPHA = 0.85                         # clipped-scale factor (sim-optimal)

_prog_cache = {}
_fn_cache = {}


# ---------------------------------------------------------------------------
def _apply_bir_wait_split(bass_mod):
    """This container's walrus rejects >1 sync-wait per instruction. Split
    extras onto inserted EventSemaphore instructions (same engine, in order).
    """
    if getattr(bass_mod.Bass, "_wait_split_applied", False):
        return
    orig = bass_mod.Bass.to_json_bytes
    ctr = [0]

    def fix_list(lst):
        out, changed = [], False
        for ins in lst:
            si = ins.get("sync_info") if isinstance(ins, dict) else None
            if not si:
                out.append(ins)
                continue
            waits = si.get("on_wait") or []
            upds = si.get("on_update") or []
            if len(waits) > 1:
                for w in waits[1:]:
                    ctr[0] += 1
                    out.append({
                        "debug": ins.get("debug", 0), "engine": ins["engine"],
                        "ins": [], "name": f"I-waitfix-{ctr[0]}",
                        "opcode": "EventSemaphore", "outs": [],
                        "sync_info": {"on_update": [], "on_wait": [w]},
                    })
                si["on_wait"] = waits[:1]
                changed = True
            out.append(ins)
            if len(upds) > 1:
                for u in upds[1:]:
                    ctr[0] += 1
                    out.append({
                        "debug": ins.get("debug", 0), "engine": ins["engine"],
                        "ins": [], "name": f"I-updfix-{ctr[0]}",
                        "opcode": "EventSemaphore", "outs": [],
                        "sync_info": {"on_update": [u], "on_wait": []},
                    })
                si["on_update"] = upds[:1]
                changed = True
        return out, changed

    def walk(o):
        if isinstance(o, dict):
            for k, v in o.items():
                if (isinstance(v, list) and v
                        and all(isinstance(e, dict) and "opcode" in e for e in v)):
                    fixed, changed = fix_list(v)
                    if changed:
                        o[k] = fixed
                    for e in o[k]:
                        walk(e)
                else:
                    walk(v)
        elif isinstance(o, list):
            for v in o:
                walk(v)

    def to_json_bytes_fixed(self):
        d = json.loads(orig(self))
        walk(d)
        return json.dumps(d).encode()

    bass_mod.Bass.to_json_bytes = to_json_bytes_fixed
    bass_mod.Bass._wait_split_applied = True


# ---------------------------------------------------------------------------
def _build_program(mask_entries, has_clsb, has_bias):
    """Build the Bass program (shared by all 8 cores).

    mask_entries: sorted tuple of (d, s) recurrence slots that need the
    data-driven carry-through lerp (d: 0 fwd / 1 bwd, s: step index).
    When no biases/masks/clsb exist, the bb input is dropped entirely.
    """
    import concourse.bass as bass
    import concourse.mybir as mybir
    import concourse.tile as tile

    _apply_bir_wait_split(bass)

    bf16 = mybir.dt.bfloat16
    f32 = mybir.dt.float32
    u8 = mybir.dt.uint8
    f8 = mybir.dt.float8e3
    AF = mybir.ActivationFunctionType
    ALU = mybir.AluOpType

    nc = bass.Bass(num_devices=NCORES)

    nmask = max(1, len(mask_entries))
    NB = 16 + 4 * nmask + (NCLS if has_clsb else 0)
    need_bb = has_bias or bool(mask_entries) or has_clsb

    # ---- DRAM I/O ----
    xq_d = nc.dram_tensor("xq", [EMB, XPK], u8, kind="ExternalInput")
    xs_d = nc.dram_tensor("xs", [128, 3], f32, kind="ExternalInput")
    wsh_d = nc.dram_tensor("wsh", [1, WSH], f8, kind="ExternalInput")
    bb_d = (nc.dram_tensor("bb", [128, NB], f32, kind="ExternalInput")
            if need_bb else None)
    out_d = nc.dram_tensor("out", [NTOK, NCLS], f32, kind="ExternalOutput")

    mask_idx = {ds: i for i, ds in enumerate(mask_entries)}

    with tile.TileContext(nc) as tc:
        with (
            tc.tile_pool(name="const", bufs=1) as cpool,
            tc.tile_pool(name="big", bufs=1) as bigpool,
            tc.tile_pool(name="state", bufs=1) as spool,
            tc.tile_pool(name="dram", bufs=1, space="DRAM") as dram,
        ):
            # ---- weight wall: fp8 shard -> DRAM AllGather -> SBUF -> bf16 ----
            inb = dram.tile([1, WSH], f8)
            outb = dram.tile([128, CTOT], f8)
            nc.gpsimd.dma_start(out=inb[:, :], in_=wsh_d[:, :])
            nc.gpsimd.collective_compute(
                "AllGather", mybir.AluOpType.bypass,
                replica_groups=[list(range(NCORES))],
                ins=[inb[:, :].opt()], outs=[outb[:, :].opt()],
            )
            wall8 = bigpool.tile([128, CTOT], f8)
            nc.gpsimd.dma_start(out=wall8[:, :], in_=outb[:, :])
            wall = bigpool.tile([128, CTOT], bf16)
            nc.vector.tensor_copy(wall[:, :], wall8[:, :])

            # ---- small constants ----
            xs_sb = cpool.tile([128, 3], f32)
            nc.gpsimd.dma_start(out=xs_sb[:, :], in_=xs_d[:, :])
            bb = None
            if need_bb:
                bb = cpool.tile([128, NB], f32)
                nc.gpsimd.dma_start(out=bb[:, :], in_=bb_d[:, :])

            # ---- embeddings: packed int5 -> bf16 dequant ----
            # Group g = token cols 8g..8g+7 (two t's x 4 examples): v0..v4 in
            # the low-5 bits of bytes p0..p4; v5..v7 ride the 15 top-3-bit
            # slots: field = v5 | v6<<5 | v7<<10, p_j top3 = field>>(3j).
            xt = []
            with tc.tile_pool(name="xqp", bufs=1) as xqp:
                for k in range(3):
                    nk = min(128, EMB - 128 * k)
                    xqt = xqp.tile([128, XPK], u8, tag=f"xq{k}", name=f"xq{k}")
                    nc.gpsimd.dma_start(
                        out=xqt[:nk, :], in_=xq_d[128 * k:128 * k + nk, :])
                    xk = bigpool.tile([128, NTOK], bf16, tag=f"xt{k}",
                                      name=f"xt{k}")
                    if nk < 128:
                        nc.vector.memset(xk[:, :], 0.0)

                    xq_nk = xqt[:nk, :]
                    xk_nk = xk[:nk, :]

                    def plane(j):
                        return bass.AP(tensor=xq_nk.tensor,
                                       offset=xq_nk.offset + j,
                                       ap=[xq_nk.ap[0]] + [[5, NG]])

                    def outp(idx):
                        return bass.AP(tensor=xk_nk.tensor,
                                       offset=xk_nk.offset + idx,
                                       ap=[xk_nk.ap[0]] + [[8, NG]])

                    sc = xs_sb[:nk, k:k + 1]

                    def dequant(idx, src):
                        nc.vector.tensor_scalar(
                            out=outp(idx), in0=src, scalar1=16.0,
                            scalar2=sc, op0=ALU.subtract, op1=ALU.mult)

                    tmp = xqp.tile([128, NG], u8, tag="tmp")
                    ta = xqp.tile([128, NG], u8, tag="ta")
                    tb = xqp.tile([128, NG], u8, tag="tb")
                    for j in range(5):
                        nc.vector.tensor_scalar(
                            out=tmp[:nk, :], in0=plane(j), scalar1=31,
                            scalar2=None, op0=ALU.bitwise_and)
                        dequant(j, tmp[:nk, :])
                    # v5 = (p0>>5) | ((p1&0x60)>>2)
                    nc.vector.tensor_scalar(
                        out=ta[:nk, :], in0=plane(0), scalar1=5,
                        scalar2=None, op0=ALU.logical_shift_right)
                    nc.vector.tensor_scalar(
                        out=tb[:nk, :], in0=plane(1), scalar1=0x60, scalar2=2,
                        op0=ALU.bitwise_and, op1=ALU.logical_shift_right)
                    nc.vector.tensor_tensor(
                        out=ta[:nk, :], in0=ta[:nk, :], in1=tb[:nk, :],
                        op=ALU.bitwise_or)
                    dequant(5, ta[:nk, :])
                    # v6 = ((p1&0x80)>>7) | ((p2&0xE0)>>4) | ((p3&0x20)>>1)
                    nc.vector.tensor_scalar(
                        out=ta[:nk, :], in0=plane(1), scalar1=0x80, scalar2=7,
                        op0=ALU.bitwise_and, op1=ALU.logical_shift_right)
                    nc.vector.tensor_scalar(
                        out=tb[:nk, :], in0=plane(2), scalar1=0xE0, scalar2=4,
                        op0=ALU.bitwise_and, op1=ALU.logical_shift_right)
                    nc.vector.tensor_tensor(
                        out=ta[:nk, :], in0=ta[:nk, :], in1=tb[:nk, :],
                        op=ALU.bitwise_or)
                    nc.vector.tensor_scalar(
                        out=tb[:nk, :], in0=plane(3), scalar1=0x20, scalar2=1,
                        op0=ALU.bitwise_and, op1=ALU.logical_shift_right)
                    nc.vector.tensor_tensor(
                        out=ta[:nk, :], in0=ta[:nk, :], in1=tb[:nk, :],
                        op=ALU.bitwise_or)
                    dequant(6, ta[:nk, :])
                    # v7 = ((p3&0xC0)>>6) | ((p4&0xE0)>>3)
                    nc.vector.tensor_scalar(
                        out=ta[:nk, :], in0=plane(3), scalar1=0xC0, scalar2=6,
                        op0=ALU.bitwise_and, op1=ALU.logical_shift_right)
                    nc.vector.tensor_scalar(
                        out=tb[:nk, :], in0=plane(4), scalar1=0xE0, scalar2=3,
                        op0=ALU.bitwise_and, op1=ALU.logical_shift_right)
                    nc.vector.tensor_tensor(
                        out=ta[:nk, :], in0=ta[:nk, :], in1=tb[:nk, :],
                        op=ALU.bitwise_or)
                    dequant(7, ta[:nk, :])
                    xt.append(xk)

            # ---- big persistent buffers ----
            zb = bigpool.tile([128, 32 * T], bf16)
            h0f = bigpool.tile([128, NTOK], bf16)
            h0b = bigpool.tile([128, NTOK], bf16)
            h1f = bigpool.tile([128, NTOK], bf16)
            h1b = bigpool.tile([128, NTOK], bf16)

            hz = spool.tile([128, 8], bf16)
            nc.vector.memset(hz[:, :], 0.0)

            def strided(tileap, offset, dims):
                return bass.AP(tensor=tileap.tensor, offset=tileap.offset + offset,
                               ap=[tileap.ap[0]] + dims)

            ident = wall[:, OFF_ID:OFF_ID + 128]

            # ================= shared phase helpers =================
            def projection(layer):
                """Compute Zb for `layer` from its inputs (XT or H0)."""
                nk = 3 if layer == 0 else 2
                woff = OFF_W0 if layer == 0 else OFF_W1
                boff = 0 if layer == 0 else 8
                with tc.tile_pool(name=f"pj{layer}", bufs=4, space="PSUM") as pjp:
                    for d in range(2):
                        for c in range(4):
                            for nb in range(4):
                                ps = pjp.tile([128, 512], f32, tag="pj")
                                s0 = 128 * nb
                                for k in range(nk):
                                    if layer == 0:
                                        src = xt[k][:, :]
                                    else:
                                        src = (h0f if k == 0 else h0b)[:, :]
                                    if d == 0:
                                        rhs = strided(src, 4 * s0,
                                                      [[4, 128], [1, 4]])
                                    else:
                                        rhs = strided(src, 4 * (511 - s0),
                                                      [[-4, 128], [1, 4]])
                                    wcol = woff + (d * nk + k) * G4 + c * 128
                                    nc.tensor.matmul(
                                        ps[:, :],
                                        wall[:, wcol:wcol + 128],
                                        rhs, start=(k == 0), stop=(k == nk - 1))
                                dst = strided(zb[:, :], 32 * s0 + 16 * d + 4 * c,
                                              [[32, 128], [1, 4]])
                                if need_bb:
                                    nc.scalar.activation(
                                        dst, ps[:, :], AF.Identity,
                                        bias=bb[:, boff + 4 * d + c:
                                                boff + 4 * d + c + 1],
                                        scale=1.0)
                                else:
                                    nc.scalar.activation(
                                        dst, ps[:, :], AF.Identity, scale=1.0)

            def recurrence(layer):
                roff = OFF_R0 if layer == 0 else OFF_R1
                Hf = h0f if layer == 0 else h1f
                Hb = h0b if layer == 0 else h1b
                with (
                    tc.tile_pool(name=f"rc{layer}", bufs=6, space="PSUM") as rcp,
                    tc.tile_pool(name=f"gt{layer}", bufs=8) as gtp,
                    tc.tile_pool(name=f"tm{layer}", bufs=8) as tmp,
                ):
                    c_state = spool.tile([128, 8], f32, tag=f"c{layer}")
                    nc.vector.memset(c_state[:, :], 0.0)
                    ps = None
                    prev_ht = None
                    for s in range(T):
                        sb = s % 16
                        if sb == 0:
                            ps = rcp.tile([128, 512], f32, tag="bank")
                            nc.tensor.matmul(
                                ps[:, :], ident,
                                zb[:, 512 * (s // 16):512 * (s // 16) + 512],
                                start=True, stop=False, skip_group_check=True)
                        for d in range(2):
                            if s == 0:
                                hprev = hz[:, 4 * d:4 * d + 4]
                            elif prev_ht is not None:
                                hprev = prev_ht[:, 4 * d:4 * d + 4]
                            elif d == 0:
                                hprev = Hf[:, 4 * (s - 1):4 * (s - 1) + 4]
                            else:
                                hprev = Hb[:, 4 * (512 - s):4 * (512 - s) + 4]
                            for c in range(4):
                                rcol = roff + d * G4 + c * 128
                                nc.tensor.matmul(
                                    ps[:, 32 * sb + 16 * d + 4 * c:
                                       32 * sb + 16 * d + 4 * c + 4],
                                    wall[:, rcol:rcol + 128],
                                    hprev, start=False, stop=False,
                                    skip_group_check=True)
                        sg = gtp.tile([128, 32], f32, tag="sg")
                        nc.scalar.activation(
                            sg[:, :], ps[:, 32 * sb:32 * sb + 32], AF.Sigmoid,
                            scale=WINV)
                        sga = sg[:, :]
                        i_ap = strided(sga, 0, [[16, 2], [1, 4]])
                        f_ap = strided(sga, 4, [[16, 2], [1, 4]])
                        g_ap = strided(sga, 8, [[16, 2], [1, 4]])
                        # i*(2g'-1) = 2*i*(g'-0.5): one fused op; the *2 folds
                        # into the final accumulate.
                        w_t = tmp.tile([128, 8], f32, tag="w")
                        nc.vector.scalar_tensor_tensor(
                            out=w_t[:, :], in0=g_ap, scalar=0.5, in1=i_ap,
                            op0=ALU.subtract, op1=ALU.mult)
                        v = tmp.tile([128, 8], f32, tag="v")
                        nc.vector.tensor_tensor(
                            out=v[:, :], in0=f_ap, in1=c_state[:, :], op=ALU.mult)
                        masked = [d for d in range(2) if (d, s) in mask_idx]
                        if not masked:
                            nc.vector.scalar_tensor_tensor(
                                out=c_state[:, :], in0=w_t[:, :], scalar=2.0,
                                in1=v[:, :], op0=ALU.mult, op1=ALU.add)
                            th = tmp.tile([128, 8], f32, tag="th")
                            nc.scalar.activation(th[:, :], c_state[:, :], AF.Tanh)
                            o_ap = strided(sga, 12, [[16, 2], [1, 4]])
                            ht = tmp.tile([128, 8], bf16, tag="ht")
                            nc.vector.tensor_tensor(
                                out=ht[:, :], in0=o_ap, in1=th[:, :],
                                op=ALU.mult)
                            nc.vector.tensor_copy(
                                Hf[:, 4 * s:4 * s + 4], ht[:, 0:4])
                            nc.vector.tensor_copy(
                                Hb[:, 4 * (511 - s):4 * (511 - s) + 4],
                                ht[:, 4:8])
                            prev_ht = ht
                        else:
                            cc = tmp.tile([128, 8], f32, tag="cc")
                            nc.vector.scalar_tensor_tensor(
                                out=cc[:, :], in0=w_t[:, :], scalar=2.0,
                                in1=v[:, :], op0=ALU.mult, op1=ALU.add)
                            # c lerp: cc_d = c_old + m*(cc_d - c_old)
                            for d in masked:
                                mi = mask_idx[(d, s)]
                                mcol = bb[:, 16 + 4 * mi:16 + 4 * mi + 4]
                                dd = tmp.tile([128, 4], f32, tag="dd")
                                nc.vector.tensor_tensor(
                                    out=dd[:, :], in0=cc[:, 4 * d:4 * d + 4],
                                    in1=c_state[:, 4 * d:4 * d + 4], op=ALU.subtract)
                                nc.vector.tensor_tensor(
                                    out=dd[:, :], in0=dd[:, :], in1=mcol, op=ALU.mult)
                                nc.vector.tensor_tensor(
                                    out=cc[:, 4 * d:4 * d + 4], in0=dd[:, :],
                                    in1=c_state[:, 4 * d:4 * d + 4], op=ALU.add)
                            nc.vector.tensor_copy(c_state[:, :], cc[:, :])
                            th = tmp.tile([128, 8], f32, tag="th")
                            nc.scalar.activation(th[:, :], c_state[:, :], AF.Tanh)
                            for d in range(2):
                                o_sl = sg[:, 16 * d + 12:16 * d + 16]
                                th_sl = th[:, 4 * d:4 * d + 4]
                                dst = (Hf[:, 4 * s:4 * s + 4] if d == 0 else
                                       Hb[:, 4 * (511 - s):4 * (511 - s) + 4])
                                if d in masked:
                                    mi = mask_idx[(d, s)]
                                    mcol = bb[:, 16 + 4 * mi:16 + 4 * mi + 4]
                                    if s == 0:
                                        hp = hz[:, 4 * d:4 * d + 4]
                                    elif d == 0:
                                        hp = Hf[:, 4 * (s - 1):4 * (s - 1) + 4]
                                    else:
                                        hp = Hb[:, 4 * (512 - s):4 * (512 - s) + 4]
                                    hn = tmp.tile([128, 4], f32, tag="hn")
                                    nc.vector.tensor_tensor(
                                        out=hn[:, :], in0=o_sl, in1=th_sl,
                                        op=ALU.mult)
                                    nc.vector.tensor_tensor(
                                        out=hn[:, :], in0=hn[:, :], in1=hp,
                                        op=ALU.subtract)
                                    nc.vector.tensor_tensor(
                                        out=hn[:, :], in0=hn[:, :], in1=mcol,
                                        op=ALU.mult)
                                    nc.vector.tensor_tensor(
                                        out=dst, in0=hn[:, :], in1=hp, op=ALU.add)
                                else:
                                    nc.vector.tensor_tensor(
                                        out=dst, in0=o_sl, in1=th_sl, op=ALU.mult)
                            prev_ht = None

            # ================= run the phases =================
            projection(0)
            recurrence(0)
            projection(1)
            recurrence(1)

            # ================= classifier + softmax =================
            with (
                tc.tile_pool(name="cls", bufs=4) as clp,
                tc.tile_pool(name="clps", bufs=4, space="PSUM") as clps,
            ):
                for tt in range(NTILE):
                    sl = slice(128 * tt, 128 * (tt + 1))
                    i0 = clp.tile([128, 128], bf16, tag="i0")
                    nc.vector.tensor_tensor(
                        out=i0[:, :], in0=h0f[:, sl], in1=h1f[:, sl], op=ALU.add)
                    i1 = clp.tile([128, 128], bf16, tag="i1")
                    nc.vector.tensor_tensor(
                        out=i1[:, :], in0=h0b[:, sl], in1=h1b[:, sl], op=ALU.add)
                    pc = clps.tile([128, NCLS], f32, tag="pc")
                    nc.tensor.matmul(pc[:, :], i0[:, :],
                                     wall[:, OFF_CLS:OFF_CLS + NCLS],
                                     start=True, stop=False)
                    nc.tensor.matmul(pc[:, :], i1[:, :],
                                     wall[:, OFF_CLS + NCLS:OFF_CLS + 2 * NCLS],
                                     start=False, stop=True)
                    ex = clp.tile([128, NCLS], f32, tag="ex")
                    if has_clsb:
                        # bb holds 16*clsb, so Exp(scale/16) yields
                        # exp(logits + clsb).
                        nc.vector.tensor_tensor(
                            out=ex[:, :], in0=pc[:, :],
                            in1=bb[:, NB - NCLS:NB], op=ALU.add)
                        nc.scalar.activation(ex[:, :], ex[:, :], AF.Exp,
                                             scale=WINV)
                    else:
                        nc.scalar.activation(ex[:, :], pc[:, :], AF.Exp,
                                             scale=WINV)
                    ssum = clp.tile([128, 1], f32, tag="ss")
                    nc.vector.tensor_reduce(
                        out=ssum[:, :], in_=ex[:, :], op=ALU.add,
                        axis=mybir.AxisListType.X)
                    rec_t = clp.tile([128, 1], f32, tag="rc")
                    nc.vector.reciprocal(rec_t[:, :], ssum[:, :])
                    sm = clp.tile([128, NCLS], f32, tag="sm")
                    nc.vector.tensor_scalar_mul(sm[:, :], ex[:, :], rec_t[:, :])
                    nc.gpsimd.dma_start(out=out_d[sl, :], in_=sm[:, :])

    return nc


# ---------------------------------------------------------------------------
def _prep_host(inputs):
    """Shard + pre-arrange all device inputs. Returns (in_maps, prog_key)."""
    ids = np.asarray(inputs["ids"])
    emb = np.asarray(inputs["emb_table"], dtype=np.float32)

    def gate2(wk):
        w = np.array(wk, dtype=np.float32, copy=True)
        w[:, 2 * UNITS:3 * UNITS] *= 2.0
        return w

    def pad_k(w, kpad):
        out = np.zeros((kpad, G4), np.float32)
        out[:w.shape[0]] = w
        return out

    # ---- weight wall [128, CTOT], stored fp8e3 scaled by WSCALE ----
    wall = np.zeros((128, CTOT), np.float32)
    w0 = np.stack([pad_k(gate2(inputs["fw0_k"]), 384),
                   pad_k(gate2(inputs["bw0_k"]), 384)])      # [2,384,512]
    wall[:, OFF_W0:OFF_R0] = (
        w0.reshape(2, 3, 128, G4).transpose(2, 0, 1, 3).reshape(128, 6 * G4))
    r0 = np.stack([gate2(inputs["fw0_r"]), gate2(inputs["bw0_r"])])
    wall[:, OFF_R0:OFF_W1] = r0.transpose(1, 0, 2).reshape(128, 2 * G4)
    w1 = np.stack([gate2(inputs["fw1_k"]), gate2(inputs["bw1_k"])])  # [2,256,512]
    wall[:, OFF_W1:OFF_R1] = (
        w1.reshape(2, 2, 128, G4).transpose(2, 0, 1, 3).reshape(128, 4 * G4))
    r1 = np.stack([gate2(inputs["fw1_r"]), gate2(inputs["bw1_r"])])
    wall[:, OFF_R1:OFF_CLS] = r1.transpose(1, 0, 2).reshape(128, 2 * G4)
    clsw = np.asarray(inputs["cls_w"], np.float32).reshape(2, 128, NCLS)
    wall[:, OFF_CLS:OFF_ID] = clsw.transpose(1, 0, 2).reshape(128, 2 * NCLS)
    wall *= WSCALE
    wall[:, OFF_ID:CTOT] = np.eye(128, dtype=np.float32)  # identity stays 1.0
    wall_bf = wall.astype(ml_dtypes.float8_e3m4)

    def bias_tile(bf, bb_):
        out = np.zeros((128, 8), np.float32)
        for d, b in enumerate((bf, bb_)):
            b = np.array(b, dtype=np.float32, copy=True)
            b[2 * UNITS:3 * UNITS] *= 2.0
            out[:, 4 * d:4 * d + 4] = b.reshape(4, 128).T
        return out

    b0 = bias_tile(inputs["fw0_b"], inputs["bw0_b"])
    b1 = bias_tile(inputs["fw1_b"], inputs["bw1_b"])
    clsb_np = np.asarray(inputs["cls_b"], np.float32)
    has_clsb = bool(np.any(clsb_np != 0))

    # ---- embeddings: host gather + transpose + packed int5 quant ----
    x_all = emb[ids]                                        # [B, T, 300] f32
    sf = np.abs(x_all).reshape(-1, EMB).max(0) * XALPHA / 15.0
    sf = np.where(sf == 0, 1.0, sf)
    xs = np.ones((128, 3), np.float32)
    for k in range(3):
        nk = min(128, EMB - 128 * k)
        xs[:nk, k] = sf[128 * k:128 * k + nk]

    mask_entry_set = set()
    per_core = []
    for c in range(NCORES):
        ids_c = ids[BL * c:BL * (c + 1)]                    # [BL, T]
        x_c = x_all[BL * c:BL * (c + 1)]                    # [BL, T, 300]
        xt_c = x_c.transpose(2, 1, 0).reshape(EMB, NTOK)    # col j = 4t+e
        u = (np.clip(np.rint(xt_c / sf[:, None]), -15, 15) + 16).astype(
            np.int32).reshape(EMB, NG, 8)
        field = u[..., 5] | (u[..., 6] << 5) | (u[..., 7] << 10)
        planes = [(u[..., j] | (((field >> (3 * j)) & 7) << 5)).astype(np.uint8)
                  for j in range(5)]
        xq_c = np.stack(planes, axis=-1).reshape(EMB, XPK)
        mask_c = (ids_c != 0)
        for e, t in zip(*np.nonzero(~mask_c)):
            mask_entry_set.add((0, int(t)))          # fwd step s = t
            mask_entry_set.add((1, int(511 - t)))    # bwd step s = 511 - t
        per_core.append((xq_c, mask_c))

    mask_entries = tuple(sorted(mask_entry_set))
    nmask = max(1, len(mask_entries))
    NB = 16 + 4 * nmask + (NCLS if has_clsb else 0)
    has_bias = bool(np.any(b0 != 0) or np.any(b1 != 0))
    need_bb = has_bias or bool(mask_entries) or has_clsb

    in_maps = []
    for c in range(NCORES):
        xq_c, mask_c = per_core[c]
        wsh_c = wall_bf[SHROWS * c:SHROWS * (c + 1), :].reshape(1, WSH)
        m = dict(xq=xq_c, xs=xs, wsh=wsh_c)
        if need_bb:
            bbt = np.zeros((128, NB), np.float32)
            # zb carries WSCALE*z, so the projection biases ride scaled too
            bbt[:, 0:8] = b0 * WSCALE
            bbt[:, 8:16] = b1 * WSCALE
            bbt[:, 16:16 + 4 * nmask] = 1.0
            for mi, (d, s) in enumerate(mask_entries):
                t = s if d == 0 else 511 - s
                bbt[:, 16 + 4 * mi:16 + 4 * mi + 4] = (
                    mask_c[:, t].astype(np.float32)[None, :])
            if has_clsb:
                bbt[:, NB - NCLS:NB] = clsb_np[None, :] * WSCALE
            m["bb"] = bbt
        in_maps.append(m)
    return in_maps, (mask_entries, has_clsb, has_bias)


# ---------------------------------------------------------------------------
def _make_fn(nc):
    """jit'd SPMD executor for the program. The donated output zero-buffers
    are created ON DEVICE once (device_put) and reused across calls — the
    kernel overwrites every output element, so their content is irrelevant
    after the first write. Returns (fn, in_names, dzeros) where dzeros are
    the device-resident trailing args."""
    import jax
    import concourse.mybir as mybir
    from concourse import bass2jax
    from jax.sharding import Mesh, PartitionSpec, NamedSharding
    from jax.experimental.shard_map import shard_map

    bass2jax.install_neuronx_cc_hook()
    partition_name = (nc.partition_id_tensor.name
                      if nc.partition_id_tensor else None)
    in_names, out_names, out_avals = [], [], []
    for alloc in nc.m.functions[0].allocations:
        if not isinstance(alloc, mybir.MemoryLocationSet):
            continue
        name = alloc.memorylocations[0].name
        if alloc.kind == "ExternalInput":
            if name != partition_name:
                in_names.append(name)
        elif alloc.kind == "ExternalOutput":
            shape = tuple(alloc.tensor_shape)
            dtype = mybir.dt.np(alloc.dtype)
            out_names.append(name)
            out_avals.append(jax.core.ShapedArray(shape, dtype))
    n_params = len(in_names)
    n_outs = len(out_avals)
    all_in_names = list(in_names) + list(out_names)
    if partition_name is not None:
        all_in_names.append(partition_name)

    def _body(*args):
        operands = list(args)
        if partition_name is not None:
            operands.append(bass2jax.partition_id_tensor())
        return tuple(bass2jax._bass_exec_p.bind(
            *operands, out_avals=tuple(out_avals), in_names=tuple(all_in_names),
            out_names=tuple(out_names), lowering_input_output_aliases=(),
            sim_require_finite=True, sim_require_nnan=True, nc=nc))

    devices = jax.devices()[:NCORES]
    mesh = Mesh(np.asarray(devices), ("core",))
    fn = jax.jit(shard_map(_body, mesh=mesh,
                           in_specs=(PartitionSpec("core"),) * (n_params + n_outs),
                           out_specs=(PartitionSpec("core"),) * n_outs,
                           check_rep=False), keep_unused=True)
    shard = NamedSharding(mesh, PartitionSpec("core"))
    dzeros = [jax.device_put(
        np.zeros((av.shape[0] * NCORES,) + tuple(av.shape[1:]), av.dtype), shard)
        for av in out_avals]
    jax.block_until_ready(dzeros)
    return fn, in_names, dzeros


def _get_fn(prog_key):
    if prog_key not in _prog_cache:
        _prog_cache[prog_key] = _build_program(*prog_key)
    nc = _prog_cache[prog_key]
    if prog_key not in _fn_cache:
        _fn_cache[prog_key] = _make_fn(nc)
    return _fn_cache[prog_key]


def make_args(in_maps, in_names):
    return [np.concatenate([np.asarray(in_maps[c][nm]) for c in range(NCORES)],
                           axis=0) for nm in in_names]


# ---------------------------------------------------------------------------
def kernel(**inputs):
    in_maps, prog_key = _prep_host(inputs)
    fn, in_names, dzeros = _get_fn(prog_key)
    args = make_args(in_maps, in_names) + dzeros
    outs = fn(*args)
    oc = np.asarray(outs[0]).reshape(NCORES, T, BL, NCLS)
    return np.ascontiguousarray(
        oc.transpose(0, 2, 1, 3).reshape(B, T, NCLS)).astype(np.float32)


# revision 30
# speedup vs baseline: 6.4857x; 1.0463x over previous
"""Trainium2 Bass kernel for a 2-layer BiLSTM text tagger.

Model (see reference): embedding gather -> BiLSTM(128) -> BiLSTM(128) with
residual -> dense(279) -> softmax. mask_zero=True semantics (state + output
carry-through at masked steps).

Sharding: data-parallel over batch, 4 examples per core on 8 cores.

The per-call cost in this environment is dominated by host->device transfer
over the axon relay (~50-90 MB/s, ~80 ms dispatch floor), so the I/O layout
is optimized for minimum bytes on the wire:
  - the embedding gather + transpose happens on HOST; the device receives
    xq [300, 2048] int8 (per-feature-scaled) per core (~0.6 MB/core),
    dequantized to bf16 on device with one DVE op per 128-feature chunk.
  - all weights (layer0/1 kernels+recurrent, classifier, identity) are
    packed into ONE [128, CTOT] bf16 "wall"; each core receives 1/8 of it
    (16 rows) and an on-device DRAM AllGather reconstructs the full wall
    (2 MB total over the wire instead of 16 MB replicated).
  - biases/masks ride in one small f32 [128, NB] tile.
  - donated output zero-buffers are created on device (jnp.zeros inside the
    jitted body), not transferred.

Device layout (per core, feature/gate dim on partitions, batch in free dim):
  XT[k]  [128, 2048] bf16  - embeddings, feature = 128k+p, col j = 4t+e
  Zb     [128, 16384] bf16 - input projections in PSUM-bank order:
                             col = 32s + 16d + 4c + e (s step, d dir, c gate
                             chunk i/f/g/o, e example). g-chunk pre-scaled by
                             2 so one Sigmoid computes i,f,o sigmoids and
                             sigma(2 z_g) (tanh via 2*sig(2x)-1).
  H*     [128, 2048] bf16  - hidden states, col = 4t + e
  Recurrence step: one identity-matmul injects 16 steps of Z into a PSUM
  bank (start=True), then per step 8 accumulating matmuls add h @ Wr per
  (dir, gate-chunk); Sigmoid reads the 32-col slice; DVE computes the cell
  update with a fused scalar_tensor_tensor for the tanh fix-up.
"""

import json

import ml_dtypes
import numpy as np

# ---------------------------------------------------------------------------
# problem constants (hardcoded per the contract)
B, T = 32, 512
EMB, UNITS, NCLS = 300, 128, 279
VOCAB = 100000
NCORES = 8
BL = B // NCORES          # 4 examples / core
NTOK = BL * T             # 2048 tokens / core
G4 = 4 * UNITS            # 512
NTILE = NTOK // 128       # 16 token tiles

# wall (packed weights) column layout. Stored as fp8e3 (e3m4) scaled by
# WSCALE=16 so the small LSTM weights sit in e3m4's normal range; the /16 is
# folded into the sigmoid/exp activation `scale` (zb and the PSUM gate banks
# carry 16x values throughout). The identity section stays 1.0 (it only
# passes 16x-scaled zb through).
OFF_W0 = 0                # [128, 2, 3, 512]  (k=2 chunk rows >=44 are zero)
OFF_R0 = OFF_W0 + 2 * 3 * G4          # 3072
OFF_W1 = OFF_R0 + 2 * G4              # 4096
OFF_R1 = OFF_W1 + 2 * 2 * G4          # 6144
OFF_CLS = OFF_R1 + 2 * G4             # 7168, [128, 2, 279]
OFF_ID = OFF_CLS + 2 * NCLS           # 7726, [128, 128] identity
CTOT = OFF_ID + 128                   # 7854
WTOT = 128 * CTOT                     # 1,005,312 elems
WSH = WTOT // NCORES                  # 125,664 elems (= 16 rows)
SHROWS = 128 // NCORES                # 16 rows per shard
WSCALE = 16.0
WINV = 1.0 / WSCALE
NG = NTOK // 8                        # 256 int5 groups (8 values / 5 bytes)
XPK = 5 * NG                          # 1280 packed cols
XALPHA = 0.85                         # clipped-scale factor (sim-optimal)

# single-blob input layout (bytes): [xq | wsh(fp8) | xs(f32)] — one PJRT
# transfer instead of three (the relay charges per-array overhead)
OFF_BXQ = 0
NB_XQ = EMB * XPK                     # 384,000
OFF_BWSH = OFF_BXQ + NB_XQ
OFF_BXS = OFF_BWSH + WSH              # wall shard is 1 B/elem
NB_BLOB = OFF_BXS + 128 * 3 * 4       # + [128,3] f32 scales

_prog_cache = {}
_fn_cache = {}


# ---------------------------------------------------------------------------
def _apply_bir_wait_split(bass_mod):
    """This container's walrus rejects >1 sync-wait per instruction. Split
    extras onto inserted EventSemaphore instructions (same engine, in order).
    """
    if getattr(bass_mod.Bass, "_wait_split_applied", False):
        return
    orig = bass_mod.Bass.to_json_bytes
    ctr = [0]

    def fix_list(lst):
        out, changed = [], False
        for ins in lst:
            si = ins.get("sync_info") if isinstance(ins, dict) else None
            if not si:
                out.append(ins)
                continue
            waits = si.get("on_wait") or []
            upds = si.get("on_update") or []
            if len(waits) > 1:
                for w in waits[1:]:
                    ctr[0] += 1
                    out.append({
                        "debug": ins.get("debug", 0), "engine": ins["engine"],
                        "ins": [], "name": f"I-waitfix-{ctr[0]}",
                        "opcode": "EventSemaphore", "outs": [],
                        "sync_info": {"on_update": [], "on_wait": [w]},
                    })
                si["on_wait"] = waits[:1]
                changed = True
            out.append(ins)
            if len(upds) > 1:
                for u in upds[1:]:
                    ctr[0] += 1
                    out.append({
                        "debug": ins.get("debug", 0), "engine": ins["engine"],
                        "ins": [], "name": f"I-updfix-{ctr[0]}",
                        "opcode": "EventSemaphore", "outs": [],
                        "sync_info": {"on_update": [u], "on_wait": []},
                    })
                si["on_update"] = upds[:1]
                changed = True
        return out, changed

    def walk(o):
        if isinstance(o, dict):
            for k, v in o.items():
                if (isinstance(v, list) and v
                        and all(isinstance(e, dict) and "opcode" in e for e in v)):
                    fixed, changed = fix_list(v)
                    if changed:
                        o[k] = fixed
                    for e in o[k]:
                        walk(e)
                else:
                    walk(v)
        elif isinstance(o, list):
            for v in o:
                walk(v)

    def to_json_bytes_fixed(self):
        d = json.loads(orig(self))
        walk(d)
        return json.dumps(d).encode()

    bass_mod.Bass.to_json_bytes = to_json_bytes_fixed
    bass_mod.Bass._wait_split_applied = True


# ---------------------------------------------------------------------------
def _build_program(mask_entries, has_clsb, has_bias):
    """Build the Bass program (shared by all 8 cores).

    mask_entries: sorted tuple of (d, s) recurrence slots that need the
    data-driven carry-through lerp (d: 0 fwd / 1 bwd, s: step index).
    When no biases/masks/clsb exist, the bb input is dropped entirely.
    """
    import concourse.bass as bass
    import concourse.mybir as mybir
    import concourse.tile as tile

    _apply_bir_wait_split(bass)

    bf16 = mybir.dt.bfloat16
    f32 = mybir.dt.float32
    u8 = mybir.dt.uint8
    f8 = mybir.dt.float8e3
    AF = mybir.ActivationFunctionType
    ALU = mybir.AluOpType

    nc = bass.Bass(num_devices=NCORES)

    nmask = max(1, len(mask_entries))
    NB = 16 + 4 * nmask + (NCLS if has_clsb else 0)
    need_bb = has_bias or bool(mask_entries) or has_clsb

    # ---- DRAM I/O ----
    blob_d = nc.dram_tensor("blob", [1, NB_BLOB], u8, kind="ExternalInput")
    bb_d = (nc.dram_tensor("bb", [128, NB], f32, kind="ExternalInput")
            if need_bb else None)
    out_d = nc.dram_tensor("out", [NTOK, NCLS], f32, kind="ExternalOutput")

    blob_base = blob_d[:, :]

    def bap(off, dims):
        return bass.AP(tensor=blob_base.tensor, offset=blob_base.offset + off,
                       ap=dims)

    mask_idx = {ds: i for i, ds in enumerate(mask_entries)}

    with tile.TileContext(nc) as tc:
        with (
            tc.tile_pool(name="const", bufs=1) as cpool,
            tc.tile_pool(name="big", bufs=1) as bigpool,
            tc.tile_pool(name="state", bufs=1) as spool,
            tc.tile_pool(name="dram", bufs=1, space="DRAM") as dram,
        ):
            # ---- weight wall: fp8 shard -> DRAM AllGather -> SBUF -> bf16 ----
            inb = dram.tile([1, WSH], f8)
            outb = dram.tile([128, CTOT], f8)
            nc.gpsimd.dma_start(
                out=inb[:, :],
                in_=bap(OFF_BWSH, [[WSH, 1], [1, WSH]]).bitcast(f8))
            nc.gpsimd.collective_compute(
                "AllGather", mybir.AluOpType.bypass,
                replica_groups=[list(range(NCORES))],
                ins=[inb[:, :].opt()], outs=[outb[:, :].opt()],
            )
            wall8 = bigpool.tile([128, CTOT], f8)
            nc.gpsimd.dma_start(out=wall8[:, :], in_=outb[:, :])
            wall = bigpool.tile([128, CTOT], bf16)
            nc.vector.tensor_copy(wall[:, :], wall8[:, :])

            # ---- small constants ----
            xs_sb = cpool.tile([128, 3], f32)
            nc.gpsimd.dma_start(
                out=xs_sb[:, :],
                in_=bap(OFF_BXS, [[12, 128], [1, 12]]).bitcast(f32))
            bb = None
            if need_bb:
                bb = cpool.tile([128, NB], f32)
                nc.gpsimd.dma_start(out=bb[:, :], in_=bb_d[:, :])

            # ---- embeddings: packed int5 -> bf16 dequant ----
            # Group g = token cols 8g..8g+7 (two t's x 4 examples): v0..v4 in
            # the low-5 bits of bytes p0..p4; v5..v7 ride the 15 top-3-bit
            # slots: field = v5 | v6<<5 | v7<<10, p_j top3 = field>>(3j).
            xt = []
            with tc.tile_pool(name="xqp", bufs=1) as xqp:
                for k in range(3):
                    nk = min(128, EMB - 128 * k)
                    xqt = xqp.tile([128, XPK], u8, tag=f"xq{k}", name=f"xq{k}")
                    nc.gpsimd.dma_start(
                        out=xqt[:nk, :],
                        in_=bap(OFF_BXQ + 128 * k * XPK,
                                [[XPK, nk], [1, XPK]]))
                    xk = bigpool.tile([128, NTOK], bf16, tag=f"xt{k}",
                                      name=f"xt{k}")
                    if nk < 128:
                        nc.vector.memset(xk[:, :], 0.0)

                    xq_nk = xqt[:nk, :]
                    xk_nk = xk[:nk, :]

                    def plane(j):
                        return bass.AP(tensor=xq_nk.tensor,
                                       offset=xq_nk.offset + j,
                                       ap=[xq_nk.ap[0]] + [[5, NG]])

                    def outp(idx):
                        return bass.AP(tensor=xk_nk.tensor,
                                       offset=xk_nk.offset + idx,
                                       ap=[xk_nk.ap[0]] + [[8, NG]])

                    sc = xs_sb[:nk, k:k + 1]

                    def dequant(idx, src):
                        nc.vector.tensor_scalar(
                            out=outp(idx), in0=src, scalar1=16.0,
                            scalar2=sc, op0=ALU.subtract, op1=ALU.mult)

                    tmp = xqp.tile([128, NG], u8, tag="tmp")
                    ta = xqp.tile([128, NG], u8, tag="ta")
                    tb = xqp.tile([128, NG], u8, tag="tb")
                    for j in range(5):
                        nc.vector.tensor_scalar(
                            out=tmp[:nk, :], in0=plane(j), scalar1=31,
                            scalar2=None, op0=ALU.bitwise_and)
                        dequant(j, tmp[:nk, :])
                    # v5 = (p0>>5) | ((p1&0x60)>>2)
                    nc.vector.tensor_scalar(
                        out=ta[:nk, :], in0=plane(0), scalar1=5,
                        scalar2=None, op0=ALU.logical_shift_right)
                    nc.vector.tensor_scalar(
                        out=tb[:nk, :], in0=plane(1), scalar1=0x60, scalar2=2,
                        op0=ALU.bitwise_and, op1=ALU.logical_shift_right)
                    nc.vector.tensor_tensor(
                        out=ta[:nk, :], in0=ta[:nk, :], in1=tb[:nk, :],
                        op=ALU.bitwise_or)
                    dequant(5, ta[:nk, :])
                    # v6 = ((p1&0x80)>>7) | ((p2&0xE0)>>4) | ((p3&0x20)>>1)
                    nc.vector.tensor_scalar(
                        out=ta[:nk, :], in0=plane(1), scalar1=0x80, scalar2=7,
                        op0=ALU.bitwise_and, op1=ALU.logical_shift_right)
                    nc.vector.tensor_scalar(
                        out=tb[:nk, :], in0=plane(2), scalar1=0xE0, scalar2=4,
                        op0=ALU.bitwise_and, op1=ALU.logical_shift_right)
                    nc.vector.tensor_tensor(
                        out=ta[:nk, :], in0=ta[:nk, :], in1=tb[:nk, :],
                        op=ALU.bitwise_or)
                    nc.vector.tensor_scalar(
                        out=tb[:nk, :], in0=plane(3), scalar1=0x20, scalar2=1,
                        op0=ALU.bitwise_and, op1=ALU.logical_shift_right)
                    nc.vector.tensor_tensor(
                        out=ta[:nk, :], in0=ta[:nk, :], in1=tb[:nk, :],
                        op=ALU.bitwise_or)
                    dequant(6, ta[:nk, :])
                    # v7 = ((p3&0xC0)>>6) | ((p4&0xE0)>>3)
                    nc.vector.tensor_scalar(
                        out=ta[:nk, :], in0=plane(3), scalar1=0xC0, scalar2=6,
                        op0=ALU.bitwise_and, op1=ALU.logical_shift_right)
                    nc.vector.tensor_scalar(
                        out=tb[:nk, :], in0=plane(4), scalar1=0xE0, scalar2=3,
                        op0=ALU.bitwise_and, op1=ALU.logical_shift_right)
                    nc.vector.tensor_tensor(
                        out=ta[:nk, :], in0=ta[:nk, :], in1=tb[:nk, :],
                        op=ALU.bitwise_or)
                    dequant(7, ta[:nk, :])
                    xt.append(xk)

            # ---- big persistent buffers ----
            zb = bigpool.tile([128, 32 * T], bf16)
            h0f = bigpool.tile([128, NTOK], bf16)
            h0b = bigpool.tile([128, NTOK], bf16)
            h1f = bigpool.tile([128, NTOK], bf16)
            h1b = bigpool.tile([128, NTOK], bf16)

            hz = spool.tile([128, 8], bf16)
            nc.vector.memset(hz[:, :], 0.0)

            def strided(tileap, offset, dims):
                return bass.AP(tensor=tileap.tensor, offset=tileap.offset + offset,
                               ap=[tileap.ap[0]] + dims)

            ident = wall[:, OFF_ID:OFF_ID + 128]

            # ================= shared phase helpers =================
            def projection(layer):
                """Compute Zb for `layer` from its inputs (XT or H0)."""
                nk = 3 if layer == 0 else 2
                woff = OFF_W0 if layer == 0 else OFF_W1
                boff = 0 if layer == 0 else 8
                with tc.tile_pool(name=f"pj{layer}", bufs=4, space="PSUM") as pjp:
                    for d in range(2):
                        for c in range(4):
                            for nb in range(4):
                                ps = pjp.tile([128, 512], f32, tag="pj")
                                s0 = 128 * nb
                                for k in range(nk):
                                    if layer == 0:
                                        src = xt[k][:, :]
                                    else:
                                        src = (h0f if k == 0 else h0b)[:, :]
                                    if d == 0:
                                        rhs = strided(src, 4 * s0,
                                                      [[4, 128], [1, 4]])
                                    else:
                                        rhs = strided(src, 4 * (511 - s0),
                                                      [[-4, 128], [1, 4]])
                                    wcol = woff + (d * nk + k) * G4 + c * 128
                                    nc.tensor.matmul(
                                        ps[:, :],
                                        wall[:, wcol:wcol + 128],
                                        rhs, start=(k == 0), stop=(k == nk - 1))
                                dst = strided(zb[:, :], 32 * s0 + 16 * d + 4 * c,
                                              [[32, 128], [1, 4]])
                                if need_bb:
                                    nc.scalar.activation(
                                        dst, ps[:, :], AF.Identity,
                                        bias=bb[:, boff + 4 * d + c:
                                                boff + 4 * d + c + 1],
                                        scale=1.0)
                                else:
                                    nc.scalar.activation(
                                        dst, ps[:, :], AF.Identity, scale=1.0)

            def recurrence(layer):
                roff = OFF_R0 if layer == 0 else OFF_R1
                Hf = h0f if layer == 0 else h1f
                Hb = h0b if layer == 0 else h1b
                with (
                    tc.tile_pool(name=f"rc{layer}", bufs=6, space="PSUM") as rcp,
                    tc.tile_pool(name=f"gt{layer}", bufs=8) as gtp,
                    tc.tile_pool(name=f"tm{layer}", bufs=8) as tmp,
                ):
                    c_state = spool.tile([128, 8], f32, tag=f"c{layer}")
                    nc.vector.memset(c_state[:, :], 0.0)
                    ps = None
                    prev_ht = None
                    for s in range(T):
                        sb = s % 16
                        if sb == 0:
                            ps = rcp.tile([128, 512], f32, tag="bank")
                            nc.tensor.matmul(
                                ps[:, :], ident,
                                zb[:, 512 * (s // 16):512 * (s // 16) + 512],
                                start=True, stop=False, skip_group_check=True)
                        for d in range(2):
                            if s == 0:
                                hprev = hz[:, 4 * d:4 * d + 4]
                            elif prev_ht is not None:
                                hprev = prev_ht[:, 4 * d:4 * d + 4]
                            elif d == 0:
                                hprev = Hf[:, 4 * (s - 1):4 * (s - 1) + 4]
                            else:
                                hprev = Hb[:, 4 * (512 - s):4 * (512 - s) + 4]
                            for c in range(4):
                                rcol = roff + d * G4 + c * 128
                                nc.tensor.matmul(
                                    ps[:, 32 * sb + 16 * d + 4 * c:
                                       32 * sb + 16 * d + 4 * c + 4],
                                    wall[:, rcol:rcol + 128],
                                    hprev, start=False, stop=False,
                                    skip_group_check=True)
                        sg = gtp.tile([128, 32], f32, tag="sg")
                        nc.scalar.activation(
                            sg[:, :], ps[:, 32 * sb:32 * sb + 32], AF.Sigmoid,
                            scale=WINV)
                        sga = sg[:, :]
                        i_ap = strided(sga, 0, [[16, 2], [1, 4]])
                        f_ap = strided(sga, 4, [[16, 2], [1, 4]])
                        g_ap = strided(sga, 8, [[16, 2], [1, 4]])
                        # i*(2g'-1) = 2*i*(g'-0.5): one fused op; the *2 folds
                        # into the final accumulate.
                        w_t = tmp.tile([128, 8], f32, tag="w")
                        nc.vector.scalar_tensor_tensor(
                            out=w_t[:, :], in0=g_ap, scalar=0.5, in1=i_ap,
                            op0=ALU.subtract, op1=ALU.mult)
                        v = tmp.tile([128, 8], f32, tag="v")
                        nc.vector.tensor_tensor(
                            out=v[:, :], in0=f_ap, in1=c_state[:, :], op=ALU.mult)
                        masked = [d for d in range(2) if (d, s) in mask_idx]
                        if not masked:
                            nc.vector.scalar_tensor_tensor(
                                out=c_state[:, :], in0=w_t[:, :], scalar=2.0,
                                in1=v[:, :], op0=ALU.mult, op1=ALU.add)
                            th = tmp.tile([128, 8], f32, tag="th")
                            nc.scalar.activation(th[:, :], c_state[:, :], AF.Tanh)
                            o_ap = strided(sga, 12, [[16, 2], [1, 4]])
                            ht = tmp.tile([128, 8], bf16, tag="ht")
                            nc.vector.tensor_tensor(
                                out=ht[:, :], in0=o_ap, in1=th[:, :],
                                op=ALU.mult)
                            nc.vector.tensor_copy(
                                Hf[:, 4 * s:4 * s + 4], ht[:, 0:4])
                            nc.vector.tensor_copy(
                                Hb[:, 4 * (511 - s):4 * (511 - s) + 4],
                                ht[:, 4:8])
                            prev_ht = ht
                        else:
                            cc = tmp.tile([128, 8], f32, tag="cc")
                            nc.vector.scalar_tensor_tensor(
                                out=cc[:, :], in0=w_t[:, :], scalar=2.0,
                                in1=v[:, :], op0=ALU.mult, op1=ALU.add)
                            # c lerp: cc_d = c_old + m*(cc_d - c_old)
                            for d in masked:
                                mi = mask_idx[(d, s)]
                                mcol = bb[:, 16 + 4 * mi:16 + 4 * mi + 4]
                                dd = tmp.tile([128, 4], f32, tag="dd")
                                nc.vector.tensor_tensor(
                                    out=dd[:, :], in0=cc[:, 4 * d:4 * d + 4],
                                    in1=c_state[:, 4 * d:4 * d + 4], op=ALU.subtract)
                                nc.vector.tensor_tensor(
                                    out=dd[:, :], in0=dd[:, :], in1=mcol, op=ALU.mult)
                                nc.vector.tensor_tensor(
                                    out=cc[:, 4 * d:4 * d + 4], in0=dd[:, :],
                                    in1=c_state[:, 4 * d:4 * d + 4], op=ALU.add)
                            nc.vector.tensor_copy(c_state[:, :], cc[:, :])
                            th = tmp.tile([128, 8], f32, tag="th")
                            nc.scalar.activation(th[:, :], c_state[:, :], AF.Tanh)
                            for d in range(2):
                                o_sl = sg[:, 16 * d + 12:16 * d + 16]
                                th_sl = th[:, 4 * d:4 * d + 4]
                                dst = (Hf[:, 4 * s:4 * s + 4] if d == 0 else
                                       Hb[:, 4 * (511 - s):4 * (511 - s) + 4])
                                if d in masked:
                                    mi = mask_idx[(d, s)]
                                    mcol = bb[:, 16 + 4 * mi:16 + 4 * mi + 4]
                                    if s == 0:
                                        hp = hz[:, 4 * d:4 * d + 4]
                                    elif d == 0:
                                        hp = Hf[:, 4 * (s - 1):4 * (s - 1) + 4]
                                    else:
                                        hp = Hb[:, 4 * (512 - s):4 * (512 - s) + 4]
                                    hn = tmp.tile([128, 4], f32, tag="hn")
                                    nc.vector.tensor_tensor(
                                        out=hn[:, :], in0=o_sl, in1=th_sl,
                                        op=ALU.mult)
                                    nc.vector.tensor_tensor(
                                        out=hn[:, :], in0=hn[:, :], in1=hp,
                                        op=ALU.subtract)
                                    nc.vector.tensor_tensor(
                                        out=hn[:, :], in0=hn[:, :], in1=mcol,
                                        op=ALU.mult)
                                    nc.vector.tensor_tensor(
                                        out=dst, in0=hn[:, :], in1=hp, op=ALU.add)
                                else:
                                    nc.vector.tensor_tensor(
                                        out=dst, in0=o_sl, in1=th_sl, op=ALU.mult)
                            prev_ht = None

            # ================= run the phases =================
            projection(0)
            recurrence(0)
            projection(1)
            recurrence(1)

            # ================= classifier + softmax =================
            with (
                tc.tile_pool(name="cls", bufs=4) as clp,
                tc.tile_pool(name="clps", bufs=4, space="PSUM") as clps,
            ):
                for tt in range(NTILE):
                    sl = slice(128 * tt, 128 * (tt + 1))
                    i0 = clp.tile([128, 128], bf16, tag="i0")
                    nc.vector.tensor_tensor(
                        out=i0[:, :], in0=h0f[:, sl], in1=h1f[:, sl], op=ALU.add)
                    i1 = clp.tile([128, 128], bf16, tag="i1")
                    nc.vector.tensor_tensor(
                        out=i1[:, :], in0=h0b[:, sl], in1=h1b[:, sl], op=ALU.add)
                    pc = clps.tile([128, NCLS], f32, tag="pc")
                    nc.tensor.matmul(pc[:, :], i0[:, :],
                                     wall[:, OFF_CLS:OFF_CLS + NCLS],
                                     start=True, stop=False)
                    nc.tensor.matmul(pc[:, :], i1[:, :],
                                     wall[:, OFF_CLS + NCLS:OFF_CLS + 2 * NCLS],
                                     start=False, stop=True)
                    ex = clp.tile([128, NCLS], f32, tag="ex")
                    if has_clsb:
                        # bb holds 16*clsb, so Exp(scale/16) yields
                        # exp(logits + clsb).
                        nc.vector.tensor_tensor(
                            out=ex[:, :], in0=pc[:, :],
                            in1=bb[:, NB - NCLS:NB], op=ALU.add)
                        nc.scalar.activation(ex[:, :], ex[:, :], AF.Exp,
                                             scale=WINV)
                    else:
                        nc.scalar.activation(ex[:, :], pc[:, :], AF.Exp,
                                             scale=WINV)
                    ssum = clp.tile([128, 1], f32, tag="ss")
                    nc.vector.tensor_reduce(
                        out=ssum[:, :], in_=ex[:, :], op=ALU.add,
                        axis=mybir.AxisListType.X)
                    rec_t = clp.tile([128, 1], f32, tag="rc")
                    nc.vector.reciprocal(rec_t[:, :], ssum[:, :])
                    sm = clp.tile([128, NCLS], f32, tag="sm")
                    nc.vector.tensor_scalar_mul(sm[:, :], ex[:, :], rec_t[:, :])
                    nc.gpsimd.dma_start(out=out_d[sl, :], in_=sm[:, :])

    return nc


# ---------------------------------------------------------------------------
def _prep_host(inputs):
    """Shard + pre-arrange all device inputs. Returns (in_maps, prog_key)."""
    ids = np.asarray(inputs["ids"])
    emb = np.asarray(inputs["emb_table"], dtype=np.float32)

    def gate2(wk):
        w = np.array(wk, dtype=np.float32, copy=True)
        w[:, 2 * UNITS:3 * UNITS] *= 2.0
        return w

    def pad_k(w, kpad):
        out = np.zeros((kpad, G4), np.float32)
        out[:w.shape[0]] = w
        return out

    # ---- weight wall [128, CTOT], stored fp8e3 scaled by WSCALE ----
    wall = np.zeros((128, CTOT), np.float32)
    w0 = np.stack([pad_k(gate2(inputs["fw0_k"]), 384),
                   pad_k(gate2(inputs["bw0_k"]), 384)])      # [2,384,512]
    wall[:, OFF_W0:OFF_R0] = (
        w0.reshape(2, 3, 128, G4).transpose(2, 0, 1, 3).reshape(128, 6 * G4))
    r0 = np.stack([gate2(inputs["fw0_r"]), gate2(inputs["bw0_r"])])
    wall[:, OFF_R0:OFF_W1] = r0.transpose(1, 0, 2).reshape(128, 2 * G4)
    w1 = np.stack([gate2(inputs["fw1_k"]), gate2(inputs["bw1_k"])])  # [2,256,512]
    wall[:, OFF_W1:OFF_R1] = (
        w1.reshape(2, 2, 128, G4).transpose(2, 0, 1, 3).reshape(128, 4 * G4))
    r1 = np.stack([gate2(inputs["fw1_r"]), gate2(inputs["bw1_r"])])
    wall[:, OFF_R1:OFF_CLS] = r1.transpose(1, 0, 2).reshape(128, 2 * G4)
    clsw = np.asarray(inputs["cls_w"], np.float32).reshape(2, 128, NCLS)
    wall[:, OFF_CLS:OFF_ID] = clsw.transpose(1, 0, 2).reshape(128, 2 * NCLS)
    wall *= WSCALE
    wall[:, OFF_ID:CTOT] = np.eye(128, dtype=np.float32)  # identity stays 1.0
    wall_bf = wall.astype(ml_dtypes.float8_e3m4)

    def bias_tile(bf, bb_):
        out = np.zeros((128, 8), np.float32)
        for d, b in enumerate((bf, bb_)):
            b = np.array(b, dtype=np.float32, copy=True)
            b[2 * UNITS:3 * UNITS] *= 2.0
            out[:, 4 * d:4 * d + 4] = b.reshape(4, 128).T
        return out

    b0 = bias_tile(inputs["fw0_b"], inputs["bw0_b"])
    b1 = bias_tile(inputs["fw1_b"], inputs["bw1_b"])
    clsb_np = np.asarray(inputs["cls_b"], np.float32)
    has_clsb = bool(np.any(clsb_np != 0))

    # ---- embeddings: host gather + transpose + packed int5 quant ----
    x_all = emb[ids]                                        # [B, T, 300] f32
    sf = np.abs(x_all).reshape(-1, EMB).max(0) * XALPHA / 15.0
    sf = np.where(sf == 0, 1.0, sf)
    xs = np.ones((128, 3), np.float32)
    for k in range(3):
        nk = min(128, EMB - 128 * k)
        xs[:nk, k] = sf[128 * k:128 * k + nk]

    mask_entry_set = set()
    per_core = []
    for c in range(NCORES):
        ids_c = ids[BL * c:BL * (c + 1)]                    # [BL, T]
        x_c = x_all[BL * c:BL * (c + 1)]                    # [BL, T, 300]
        xt_c = x_c.transpose(2, 1, 0).reshape(EMB, NTOK)    # col j = 4t+e
        u = (np.clip(np.rint(xt_c / sf[:, None]), -15, 15) + 16).astype(
            np.int32).reshape(EMB, NG, 8)
        field = u[..., 5] | (u[..., 6] << 5) | (u[..., 7] << 10)
        planes = [(u[..., j] | (((field >> (3 * j)) & 7) << 5)).astype(np.uint8)
                  for j in range(5)]
        xq_c = np.stack(planes, axis=-1).reshape(EMB, XPK)
        mask_c = (ids_c != 0)
        for e, t in zip(*np.nonzero(~mask_c)):
            mask_entry_set.add((0, int(t)))          # fwd step s = t
            mask_entry_set.add((1, int(511 - t)))    # bwd step s = 511 - t
        per_core.append((xq_c, mask_c))

    mask_entries = tuple(sorted(mask_entry_set))
    nmask = max(1, len(mask_entries))
    NB = 16 + 4 * nmask + (NCLS if has_clsb else 0)
    has_bias = bool(np.any(b0 != 0) or np.any(b1 != 0))
    need_bb = has_bias or bool(mask_entries) or has_clsb

    xs_bytes = np.ascontiguousarray(xs).view(np.uint8).reshape(-1)
    in_maps = []
    for c in range(NCORES):
        xq_c, mask_c = per_core[c]
        wsh_c = wall_bf[SHROWS * c:SHROWS * (c + 1), :].reshape(-1)
        blob = np.concatenate([
            np.ascontiguousarray(xq_c).reshape(-1),
            wsh_c.view(np.uint8),
            xs_bytes,
        ])[None, :]
        assert blob.shape[1] == NB_BLOB, blob.shape
        m = dict(blob=blob)
        if need_bb:
            bbt = np.zeros((128, NB), np.float32)
            # zb carries WSCALE*z, so the projection biases ride scaled too
            bbt[:, 0:8] = b0 * WSCALE
            bbt[:, 8:16] = b1 * WSCALE
            bbt[:, 16:16 + 4 * nmask] = 1.0
            for mi, (d, s) in enumerate(mask_entries):
                t = s if d == 0 else 511 - s
                bbt[:, 16 + 4 * mi:16 + 4 * mi + 4] = (
                    mask_c[:, t].astype(np.float32)[None, :])
            if has_clsb:
                bbt[:, NB - NCLS:NB] = clsb_np[None, :] * WSCALE
            m["bb"] = bbt
        in_maps.append(m)
    return in_maps, (mask_entries, has_clsb, has_bias)


# ---------------------------------------------------------------------------
def _make_fn(nc):
    """jit'd SPMD executor for the program. The donated output zero-buffers
    are created ON DEVICE once (device_put) and reused across calls — the
    kernel overwrites every output element, so their content is irrelevant
    after the first write. Returns (fn, in_names, dzeros) where dzeros are
    the device-resident trailing args."""
    import jax
    import concourse.mybir as mybir
    from concourse import bass2jax
    from jax.sharding import Mesh, PartitionSpec, NamedSharding
    from jax.experimental.shard_map import shard_map

    bass2jax.install_neuronx_cc_hook()
    partition_name = (nc.partition_id_tensor.name
                      if nc.partition_id_tensor else None)
    in_names, out_names, out_avals = [], [], []
    for alloc in nc.m.functions[0].allocations:
        if not isinstance(alloc, mybir.MemoryLocationSet):
            continue
        name = alloc.memorylocations[0].name
        if alloc.kind == "ExternalInput":
            if name != partition_name:
                in_names.append(name)
        elif alloc.kind == "ExternalOutput":
            shape = tuple(alloc.tensor_shape)
            dtype = mybir.dt.np(alloc.dtype)
            out_names.append(name)
            out_avals.append(jax.core.ShapedArray(shape, dtype))
    n_params = len(in_names)
    n_outs = len(out_avals)
    all_in_names = list(in_names) + list(out_names)
    if partition_name is not None:
        all_in_names.append(partition_name)

    def _body(*args):
        operands = list(args)
        if partition_name is not None:
            operands.append(bass2jax.partition_id_tensor())
        return tuple(bass2jax._bass_exec_p.bind(
            *operands, out_avals=tuple(out_avals), in_names=tuple(all_in_names),
            out_names=tuple(out_names), lowering_input_output_aliases=(),
            sim_require_finite=True, sim_require_nnan=True, nc=nc))

    devices = jax.devices()[:NCORES]
    mesh = Mesh(np.asarray(devices), ("core",))
    fn = jax.jit(shard_map(_body, mesh=mesh,
                           in_specs=(PartitionSpec("core"),) * (n_params + n_outs),
                           out_specs=(PartitionSpec("core"),) * n_outs,
                           check_rep=False), keep_unused=True)
    shard = NamedSharding(mesh, PartitionSpec("core"))
    dzeros = [jax.device_put(
        np.zeros((av.shape[0] * NCORES,) + tuple(av.shape[1:]), av.dtype), shard)
        for av in out_avals]
    jax.block_until_ready(dzeros)
    return fn, in_names, dzeros


def _get_fn(prog_key):
    if prog_key not in _prog_cache:
        _prog_cache[prog_key] = _build_program(*prog_key)
    nc = _prog_cache[prog_key]
    if prog_key not in _fn_cache:
        _fn_cache[prog_key] = _make_fn(nc)
    return _fn_cache[prog_key]


def make_args(in_maps, in_names):
    return [np.concatenate([np.asarray(in_maps[c][nm]) for c in range(NCORES)],
                           axis=0) for nm in in_names]


# ---------------------------------------------------------------------------
def kernel(**inputs):
    in_maps, prog_key = _prep_host(inputs)
    fn, in_names, dzeros = _get_fn(prog_key)
    args = make_args(in_maps, in_names) + dzeros
    outs = fn(*args)
    oc = np.asarray(outs[0]).reshape(NCORES, T, BL, NCLS)
    return np.ascontiguousarray(
        oc.transpose(0, 2, 1, 3).reshape(B, T, NCLS)).astype(np.float32)
